# revision 16
# baseline (speedup 1.0000x reference)
"""Trainium2 Bass kernel for nn_HMHA (heterogeneous multi-head attention).

Reference semantics (B=32, N=1024, D=128, H=8, K=16, S=21 stations, T=1003 tasks):
  - 7 per-head projections of q/h slices, three attention blocks
    (task->task, task->station, station->task), all softmaxed over keys,
    combined and projected by W_out.

Sharding: data-parallel over batch across 8 cores (4 batches/core).

v2 layout (per core, per batch), all PE inputs bf16:
  - Heads split into two buffers: A = heads 0-3, B = heads 4-7, head g of a
    buffer at 32-aligned partition band 32g (PE tile_position row/col bands).
  - Flat projections: kt/q1/q2 [128(4hx32band), N] via single [128,128] flat
    weight matmuls (cols 0:21 use the charge/station weights, 21: the task
    weights, matching position-dependent projection in the reference).
  - tt-scores: row-tiled (32x128 mode) matmuls, 2 heads concurrent,
    psum [128keys, N] -> exp (scalar ACT, scale 0.25) -> es bf16 SBUF.
  - ts-scores + station AV: 32x32 diagonal tiles, 4 heads packed per psum.
  - AV: col-tiled (128x32 mode), 4 heads packed into [128, 512] psum halves,
    Vaug carries a ones-slot per head -> row 32g+16 = softmax denominator.
  - Normalization: denominators DMAd to [1,N] tiles, reciprocal on DVE,
    gpsimd partition_broadcast + DVE band copy, all-SBUF band-aligned muls.
  - Output: heads [128(8hx16), N] bf16 x flat W_out [128, 128] accumulated
    over the two buffers per 128-col n-tile.
"""
import numpy as np

NUM_STATION = 20
S = NUM_STATION + 1          # 21
H = 8
D = 128
K = 16
E = 128
N = 1024
B = 32
NCORES = 8
BPC = B // NCORES            # 4 batches per core
NORM = 0.25                  # 1/sqrt(16)

_CACHE = {}


def _build():
    import concourse.bass as bass
    import concourse.tile as tile
    from concourse import bacc, mybir

    F32 = mybir.dt.float32
    BF16 = mybir.dt.bfloat16
    EXP = mybir.ActivationFunctionType.Exp

    nc = bacc.Bacc("TRN2", target_bir_lowering=False, debug=False,
                   num_devices=NCORES)

    qT_d = nc.dram_tensor("qT", [BPC, D, N], F32, kind="ExternalInput").ap()
    hT_d = nc.dram_tensor("hT", [BPC, D, N], F32, kind="ExternalInput").ap()
    wnames = ["W_query_custom", "W_query_custom_1", "W_key_custom",
              "W_val_custom", "W_query_charge_1", "W_key_charge",
              "W_val_charge"]
    w_d = {n: nc.dram_tensor(n, [H, D, K], F32, kind="ExternalInput").ap()
           for n in wnames}
    wout_d = nc.dram_tensor("W_out", [H, K, E], F32, kind="ExternalInput").ap()
    out_d = nc.dram_tensor("out", [BPC, N, E], F32, kind="ExternalOutput").ap()

    with tile.TileContext(nc) as tc:
        with tc.tile_pool(name="const", bufs=1) as const, \
             tc.tile_pool(name="sb", bufs=1) as sb, \
             tc.tile_pool(name="esp", bufs=1) as esp, \
             tc.tile_pool(name="normp", bufs=1) as normp, \
             tc.tile_pool(name="ps", bufs=1, space="PSUM") as ps:

            # psum rotation over 3 two-bank slots
            scn = [0]
            def sc_tile(shape, nm):
                t = ps.tile(shape, F32, name=nm, tag=f"sc{scn[0] % 3}")
                scn[0] += 1
                return t

            # ================= weights (once per core) =================
            def flat_w(src, nm):
                tiles = []
                for X in range(2):
                    stg = const.tile([128, 128], F32, name=f"stg_{nm}{X}",
                                     tag="wstg", bufs=2)
                    nc.vector.memset(stg[:], 0.0)
                    for g in range(4):
                        nc.sync.dma_start(stg[:, 32 * g:32 * g + K],
                                          w_d[src][4 * X + g])
                    t = const.tile([128, 128], BF16, name=f"{nm}{X}")
                    nc.vector.tensor_copy(t[:], stg[:])
                    tiles.append(t)
                return tiles

            WK = flat_w("W_key_custom", "wk")
            WKC = flat_w("W_key_charge", "wkc")
            WQ1 = flat_w("W_query_custom_1", "wq1")
            WQC1 = flat_w("W_query_charge_1", "wqc1")
            WQ2 = flat_w("W_query_custom", "wq2")

            WO = []
            for X in range(2):
                stg = const.tile([128, 128], F32, name=f"wostg{X}", tag="wstg", bufs=2)
                nc.vector.memset(stg[:], 0.0)
                for g in range(4):
                    nc.sync.dma_start(stg[32 * g:32 * g + K, :], wout_d[4 * X + g])
                t = const.tile([128, 128], BF16, name=f"wo{X}")
                nc.vector.tensor_copy(t[:], stg[:])
                WO.append(t)

            def val_w(wname, nm):
                stg = const.tile([128, 136], F32, name=f"stg_{nm}", tag="wstgv", bufs=2)
                nc.vector.memset(stg[:], 0.0)
                for h in range(H):
                    nc.sync.dma_start(stg[:, 17 * h:17 * h + K], w_d[wname][h])
                t = const.tile([128, 136], BF16, name=nm)
                nc.vector.tensor_copy(t[:], stg[:])
                return t

            WV = val_w("W_val_custom", "wv")
            WVC = val_w("W_val_charge", "wvc")

            state = {}

            # ---------------- Phase A: loads, values, projections (128x128)
            def phase_A(b):
                st = {}
                qTf = sb.tile([128, N], F32, name=f"qTf{b}", tag="qTf")
                nc.sync.dma_start(qTf[:], qT_d[b])
                hTf = sb.tile([128, N], F32, name=f"hTf{b}", tag="hTf")
                nc.sync.dma_start(hTf[:], hT_d[b])
                qTb = sb.tile([128, N], BF16, name=f"qTb{b}", tag="qTb")
                nc.vector.tensor_copy(qTb[:], qTf[:])
                hTb = sb.tile([128, N], BF16, name=f"hTb{b}", tag="hTb")
                nc.vector.tensor_copy(hTb[:], hTf[:])

                # values Vaug[j]: [128, 160] bf16 (136 data + 24 zero pad),
                # ones at col 17h+16
                va = []
                for j in range(8):
                    pv = sc_tile([128, 136], f"pv{b}_{j}")
                    nc.tensor.matmul(pv[:], hTb[:, 128 * j:128 * j + 128], WV[:],
                                     start=True, stop=True)
                    v = sb.tile([128, 160], BF16, name=f"va{b}_{j}", tag=f"va{j}")
                    nc.vector.tensor_copy(v[:, 0:136], pv[:])
                    nc.vector.memset(v[:, 136:160], 0.0)
                    v3 = v[:, 0:136].rearrange("p (h s) -> p h s", h=H)
                    nc.vector.memset(v3[:, :, K:K + 1], 1.0)
                    va.append(v)
                pvs = sc_tile([128, 136], f"pvs{b}")
                nc.tensor.matmul(pvs[0:S, :], hTb[:, 0:S], WVC[:],
                                 start=True, stop=True)
                vst4 = sb.tile([128, 160], BF16, name=f"vst4{b}", tag="vst4")
                nc.vector.tensor_copy(vst4[0:S, 0:136], pvs[0:S, :])
                nc.vector.memset(vst4[0:S, 136:160], 0.0)
                vst3 = vst4[0:S, 0:136].rearrange("p (h s) -> p h s", h=H)
                nc.vector.memset(vst3[:, :, K:K + 1], 1.0)
                for g in range(1, 4):
                    nc.vector.tensor_copy(vst4[32 * g:32 * g + S, :], vst4[0:S, :])

                kt, q1, q2 = [], [], []
                for X in range(2):
                    pk = sc_tile([128, N], f"pk{b}_{X}")
                    nc.tensor.matmul(pk[:, 0:S], WKC[X][:], hTb[:, 0:S],
                                     start=True, stop=True)
                    nc.tensor.matmul(pk[:, S:512], WK[X][:], hTb[:, S:512],
                                     start=True, stop=True)
                    nc.tensor.matmul(pk[:, 512:N], WK[X][:], hTb[:, 512:N],
                                     start=True, stop=True)
                    k_ = sb.tile([128, N], BF16, name=f"kt{b}_{X}", tag=f"kt{X}")
                    nc.vector.tensor_copy(k_[:], pk[:])
                    kt.append(k_)
                    p1 = sc_tile([128, N], f"p1{b}_{X}")
                    nc.tensor.matmul(p1[:, 0:S], WQC1[X][:], qTb[:, 0:S],
                                     start=True, stop=True)
                    nc.tensor.matmul(p1[:, S:512], WQ1[X][:], qTb[:, S:512],
                                     start=True, stop=True)
                    nc.tensor.matmul(p1[:, 512:N], WQ1[X][:], qTb[:, 512:N],
                                     start=True, stop=True)
                    q1_ = sb.tile([128, N], BF16, name=f"q1{b}_{X}", tag=f"q1{X}")
                    nc.vector.tensor_copy(q1_[:], p1[:])
                    q1.append(q1_)
                    p2 = sc_tile([128, N], f"p2{b}_{X}")
                    nc.tensor.matmul(p2[:, 0:512], WQ2[X][:], qTb[:, 0:512],
                                     start=True, stop=True)
                    nc.tensor.matmul(p2[:, 512:N], WQ2[X][:], qTb[:, 512:N],
                                     start=True, stop=True)
                    q2_ = sb.tile([128, N], BF16, name=f"q2{b}_{X}", tag=f"q2{X}")
                    nc.vector.tensor_copy(q2_[:], p2[:])
                    q2.append(q2_)
                st["kt"], st["q1"], st["q2"] = kt, q1, q2
                st["va"], st["vst4"] = va, vst4
                return st

            # ---------------- Phase B(X): tt-scores (32x128), exp on scalar
            def phase_B(b, st, X):
                es = st.setdefault("es", {})
                kt, q1 = st["kt"], st["q1"]
                for j in range(8):
                    for g in range(4):
                        h = 4 * X + g
                        stp = sc_tile([128, N], f"st{b}_{h}_{j}")
                        lhs = kt[X][32 * g:32 * g + K, 128 * j:128 * j + 128]
                        nc.tensor.matmul(stp[:, 0:512], lhs,
                                         q1[X][32 * g:32 * g + K, 0:512],
                                         start=True, stop=True,
                                         tile_position=(32 * g, 0))
                        nc.tensor.matmul(stp[:, 512:N], lhs,
                                         q1[X][32 * g:32 * g + K, 512:N],
                                         start=True, stop=True,
                                         tile_position=(32 * g, 0))
                        e_ = esp.tile([128, N], BF16, name=f"es{b}_{h}_{j}",
                                      tag=f"es{h}_{j}")
                        nc.scalar.activation(e_[:], stp[:], EXP, scale=NORM)
                        if j == 0:
                            nc.vector.memset(e_[0:S, :], 0.0)
                        es[(h, j)] = e_

            # ------- Phase TS(X): ts-scores + station AV + s-side norm prep
            def phase_TS(b, st, X):
                kt, q2, vst4 = st["kt"], st["q2"], st["vst4"]
                ps2 = sc_tile([128, N], f"ps2{b}_{X}")
                for g in range(4):
                    lhs2 = kt[X][32 * g:32 * g + K, 0:S]
                    for half in range(2):
                        nc.tensor.matmul(
                            ps2[32 * g:32 * g + S, 512 * half:512 * half + 512],
                            lhs2,
                            q2[X][32 * g:32 * g + K, 512 * half:512 * half + 512],
                            start=True, stop=True,
                            tile_position=(32 * g, 32 * g))
                e2 = sb.tile([128, N], BF16, name=f"es2{b}_{X}", tag="es2")
                nc.scalar.activation(e2[:], ps2[:], EXP, scale=NORM)
                rp = sb.tile([128, N], F32, name=f"rawpts{b}_{X}", tag=f"rpts{X}")
                for half in range(2):
                    pts = ps.tile([128, 512], F32, name=f"pts{b}_{X}_{half}",
                                  tag="pts")
                    for g in range(4):
                        h = 4 * X + g
                        nc.tensor.matmul(
                            pts[32 * g:32 * g + 32, :],
                            vst4[32 * g:32 * g + S, 17 * h:17 * h + 32],
                            e2[32 * g:32 * g + S, 512 * half:512 * half + 512],
                            start=True, stop=True,
                            tile_position=(32 * g, 32 * g))
                    nc.vector.tensor_copy(rp[:, 512 * half:512 * half + 512],
                                          pts[:])
                st.setdefault("rawpts", {})[X] = rp
                # s-side denominators -> rbs bands -> reciprocal
                rbs = sb.tile([128, N], F32, name=f"rbs{b}_{X}", tag="rbs")
                if b == 0 and X == 0:
                    nc.vector.memset(rbs[:], 1.0)
                for g in range(4):
                    srow = normp.tile([1, N], F32, name=f"srs{b}_{X}_{g}",
                                      tag="srow", bufs=2)
                    nc.sync.dma_start(srow[:], rp[32 * g + K:32 * g + 17, :])
                    rb16 = normp.tile([16, N], F32, name=f"rb16s{b}_{X}_{g}",
                                      tag="rb16")
                    nc.gpsimd.partition_broadcast(rb16[:], srow[:])
                    nc.sync.dma_start(rbs[32 * g:32 * g + K, :], rb16[:])
                nc.vector.reciprocal_approx_fast(rbs[:], rbs[:])
                st.setdefault("rbs", {})[X] = rbs

            # ---------------- Phase C(X): AV (128x32 col tiling)
            def phase_C(b, st, X):
                va, es = st["va"], st["es"]
                ra = sb.tile([128, N], F32, name=f"rawav{b}_{X}", tag=f"rav{X}")
                for half in range(2):
                    pav = ps.tile([128, 512], F32, name=f"pav{b}_{X}_{half}",
                                  tag="pav")
                    for j in range(8):
                        for g in range(4):
                            h = 4 * X + g
                            nc.tensor.matmul(
                                pav[32 * g:32 * g + 32, :],
                                va[j][:, 17 * h:17 * h + 32],
                                es[(h, j)][:, 512 * half:512 * half + 512],
                                start=(j == 0), stop=(j == 7),
                                tile_position=(0, 32 * g),
                                skip_group_check=True)
                    nc.vector.tensor_copy(ra[:, 512 * half:512 * half + 512],
                                          pav[:])
                st.setdefault("rawav", {})[X] = ra

            # ------- Phase NT(X): t-side norm + heads assembly (no PE)
            def phase_NT(b, st, X):
                ra, rp = st["rawav"][X], st["rawpts"][X]
                rbs = st["rbs"][X]
                rbt = sb.tile([128, N], F32, name=f"rbt{b}_{X}", tag="rbt")
                if b == 0 and X == 0:
                    nc.vector.memset(rbt[:], 1.0)
                for g in range(4):
                    srow = normp.tile([1, N], F32, name=f"srt{b}_{X}_{g}",
                                      tag="srow", bufs=2)
                    nc.sync.dma_start(srow[:], ra[32 * g + K:32 * g + 17, :])
                    rb16 = normp.tile([16, N], F32, name=f"rb16t{b}_{X}_{g}",
                                      tag="rb16")
                    nc.gpsimd.partition_broadcast(rb16[:], srow[:])
                    nc.sync.dma_start(rbt[32 * g:32 * g + K, :], rb16[:])
                nc.vector.reciprocal_approx_fast(rbt[:], rbt[:])
                tmp = sb.tile([128, N], BF16, name=f"tmp{b}_{X}", tag="tmp")
                hx = sb.tile([128, N], BF16, name=f"heads{b}_{X}",
                             tag=f"heads{X}")
                nc.vector.tensor_mul(hx[:], ra[:], rbt[:])
                nc.vector.tensor_mul(tmp[:, S:N], rp[:, S:N], rbs[:, S:N])
                nc.vector.tensor_add(hx[:, S:N], hx[:, S:N], tmp[:, S:N])
                st.setdefault("heads", {})[X] = hx

            # ---------------- Phase D: output projection (128x128)
            def phase_D(b, st):
                heads = st["heads"]
                for nt in range(8):
                    po = sc_tile([128, 128], f"po{b}_{nt}")
                    nc.tensor.matmul(po[:], heads[0][:, 128 * nt:128 * nt + 128],
                                     WO[0][:], start=True, stop=False,
                                     skip_group_check=True)
                    nc.tensor.matmul(po[:], heads[1][:, 128 * nt:128 * nt + 128],
                                     WO[1][:], start=False, stop=True,
                                     skip_group_check=True)
                    ot = normp.tile([128, 128], F32, name=f"ot{b}_{nt}", tag="ot")
                    nc.vector.tensor_copy(ot[:], po[:])
                    nc.sync.dma_start(out_d[b, 128 * nt:128 * nt + 128, :], ot[:])

            # ---------------- software-pipelined emission
            # Scalar continuity: ..., es(b) x64, es2(b+1) x2, es(b+1) x64.
            # D(b-1) is emitted after B(b, 0) so the next batch's scores are
            # not blocked behind the NT(b-1, 1) normalization tail.
            states = {}
            states[0] = phase_A(0)
            phase_TS(0, states[0], 0)
            phase_TS(0, states[0], 1)
            for b in range(BPC):
                st = states[b]
                phase_B(b, st, 0)
                if b > 0:
                    phase_D(b - 1, states[b - 1])
                    del states[b - 1]
                phase_B(b, st, 1)
                phase_C(b, st, 0)
                if b + 1 < BPC:
                    states[b + 1] = phase_A(b + 1)
                phase_NT(b, st, 0)
                phase_C(b, st, 1)
                phase_NT(b, st, 1)
                if b + 1 < BPC:
                    phase_TS(b + 1, states[b + 1], 0)
                    phase_TS(b + 1, states[b + 1], 1)
            phase_D(BPC - 1, states[BPC - 1])

    nc.compile()
    return nc


def _get_nc():
    if "nc" not in _CACHE:
        _CACHE["nc"] = _build()
    return _CACHE["nc"]


def _kernel_jax(q, h, Ws):
    """Batch-sharded (data-parallel) attention on the 8 NeuronCores via pmap."""
    import jax, jax.numpy as jnp
    S_ = S
    NORMc = np.float32(NORM)

    def one_shard(q, h, W_query_custom, W_query_custom_1, W_key_custom,
                  W_val_custom, W_query_charge_1, W_key_charge, W_val_charge,
                  W_out):
        h_st, h_tk = h[:, :S_], h[:, S_:]
        q_st, q_tk = q[:, :S_], q[:, S_:]
        proj = lambda x, W: jnp.einsum('bnd,hdk->hbnk', x, W)
        K_c = proj(h_tk, W_key_custom)
        V_c = proj(h_tk, W_val_custom)
        K_s = proj(h_st, W_key_charge)
        V_s = proj(h_st, W_val_charge)
        Q_tt = proj(q_tk, W_query_custom_1)
        A_tt = jax.nn.softmax(NORMc * jnp.einsum('hbqk,hbtk->hbqt', Q_tt, K_c), axis=-1)
        heads_t = jnp.einsum('hbqt,hbtk->hbqk', A_tt, V_c)
        Q_ts = proj(q_tk, W_query_custom)
        A_ts = jax.nn.softmax(NORMc * jnp.einsum('hbqk,hbsk->hbqs', Q_ts, K_s), axis=-1)
        heads_t = heads_t + jnp.einsum('hbqs,hbsk->hbqk', A_ts, V_s)
        Q_st = proj(q_st, W_query_charge_1)
        A_st = jax.nn.softmax(NORMc * jnp.einsum('hbqk,hbtk->hbqt', Q_st, K_c), axis=-1)
        heads_s = jnp.einsum('hbqt,hbtk->hbqk', A_st, V_c)
        heads = jnp.concatenate([heads_s, heads_t], axis=2)
        return jnp.einsum('hbnk,hke->bne', heads, W_out)

    if "pmap_fn" not in _CACHE:
        _CACHE["pmap_fn"] = jax.pmap(one_shard, axis_name="i")
    f = _CACHE["pmap_fn"]
    qs = q.reshape(NCORES, BPC, N, D)
    hs = h.reshape(NCORES, BPC, N, D)
    wkey = tuple(w.tobytes()[:64] for w in Ws)
    if _CACHE.get("wkey") != wkey:
        _CACHE["wrep"] = [jax.device_put_replicated(jnp.asarray(w), jax.devices()[:NCORES])
                          for w in Ws]
        _CACHE["wkey"] = wkey
    out = f(qs, hs, *_CACHE["wrep"])
    return np.asarray(out).reshape(B, N, E)


USE_BASS = True


def kernel(q, h, W_query_custom, W_query_custom_1, W_key_custom, W_val_custom,
           W_query_charge_1, W_key_charge, W_val_charge, W_out, _trace=False):
    if not USE_BASS:
        Ws = [np.asarray(w, np.float32) for w in
              (W_query_custom, W_query_custom_1, W_key_custom, W_val_custom,
               W_query_charge_1, W_key_charge, W_val_charge, W_out)]
        return _kernel_jax(np.asarray(q, np.float32), np.asarray(h, np.float32), Ws)
    return _kernel_bass(q, h, W_query_custom, W_query_custom_1, W_key_custom,
                        W_val_custom, W_query_charge_1, W_key_charge,
                        W_val_charge, W_out, _trace)


def _kernel_bass(q, h, W_query_custom, W_query_custom_1, W_key_custom, W_val_custom,
                 W_query_charge_1, W_key_charge, W_val_charge, W_out, _trace=False):
    from concourse.bass_utils import run_bass_kernel_spmd

    nc = _get_nc()
    qT = np.ascontiguousarray(np.asarray(q, dtype=np.float32).transpose(0, 2, 1))
    hT = np.ascontiguousarray(np.asarray(h, dtype=np.float32).transpose(0, 2, 1))
    ws = {
        "W_query_custom": W_query_custom, "W_query_custom_1": W_query_custom_1,
        "W_key_custom": W_key_custom, "W_val_custom": W_val_custom,
        "W_query_charge_1": W_query_charge_1, "W_key_charge": W_key_charge,
        "W_val_charge": W_val_charge, "W_out": W_out,
    }
    ws = {k: np.ascontiguousarray(np.asarray(v, dtype=np.float32))
          for k, v in ws.items()}
    in_maps = []
    for c in range(NCORES):
        m = {"qT": qT[c * BPC:(c + 1) * BPC], "hT": hT[c * BPC:(c + 1) * BPC]}
        m.update(ws)
        in_maps.append(m)
    res = run_bass_kernel_spmd(nc, in_maps, core_ids=list(range(NCORES)),
                               trace=_trace)
    out = np.concatenate([res.results[c]["out"] for c in range(NCORES)], axis=0)
    if _trace:
        _CACHE["last_results"] = res
    return out


# revision 17
# speedup vs baseline: 1.0605x; 1.0605x over previous
"""Trainium2 Bass kernel for nn_HMHA (heterogeneous multi-head attention).

Reference semantics (B=32, N=1024, D=128, H=8, K=16, S=21 stations, T=1003 tasks):
  - 7 per-head projections of q/h slices, three attention blocks
    (task->task, task->station, station->task), all softmaxed over keys,
    combined and projected by W_out.

Sharding: data-parallel over batch across 8 cores (4 batches/core).

v2 layout (per core, per batch), all PE inputs bf16:
  - Heads split into two buffers: A = heads 0-3, B = heads 4-7, head g of a
    buffer at 32-aligned partition band 32g (PE tile_position row/col bands).
  - Flat projections: kt/q1/q2 [128(4hx32band), N] via single [128,128] flat
    weight matmuls (cols 0:21 use the charge/station weights, 21: the task
    weights, matching position-dependent projection in the reference).
  - tt-scores: row-tiled (32x128 mode) matmuls, 2 heads concurrent,
    psum [128keys, N] -> exp (scalar ACT, scale 0.25) -> es bf16 SBUF.
  - ts-scores + station AV: 32x32 diagonal tiles, 4 heads packed per psum.
  - AV: col-tiled (128x32 mode), 4 heads packed into [128, 512] psum halves,
    Vaug carries a ones-slot per head -> row 32g+16 = softmax denominator.
  - Normalization: denominators DMAd to [1,N] tiles, reciprocal on DVE,
    gpsimd partition_broadcast + DVE band copy, all-SBUF band-aligned muls.
  - Output: heads [128(8hx16), N] bf16 x flat W_out [128, 128] accumulated
    over the two buffers per 128-col n-tile.
"""
import numpy as np

NUM_STATION = 20
S = NUM_STATION + 1          # 21
H = 8
D = 128
K = 16
E = 128
N = 1024
B = 32
NCORES = 8
BPC = B // NCORES            # 4 batches per core
NORM = 0.25                  # 1/sqrt(16)

_CACHE = {}


def _build():
    import concourse.bass as bass
    import concourse.tile as tile
    from concourse import bacc, mybir

    F32 = mybir.dt.float32
    BF16 = mybir.dt.bfloat16
    EXP = mybir.ActivationFunctionType.Exp

    nc = bacc.Bacc("TRN2", target_bir_lowering=False, debug=False,
                   num_devices=NCORES)

    qT_d = nc.dram_tensor("qT", [BPC, D, N], F32, kind="ExternalInput").ap()
    hT_d = nc.dram_tensor("hT", [BPC, D, N], F32, kind="ExternalInput").ap()
    wnames = ["W_query_custom", "W_query_custom_1", "W_key_custom",
              "W_val_custom", "W_query_charge_1", "W_key_charge",
              "W_val_charge"]
    w_d = {n: nc.dram_tensor(n, [H, D, K], F32, kind="ExternalInput").ap()
           for n in wnames}
    wout_d = nc.dram_tensor("W_out", [H, K, E], F32, kind="ExternalInput").ap()
    out_d = nc.dram_tensor("out", [BPC, N, E], F32, kind="ExternalOutput").ap()

    with tile.TileContext(nc) as tc:
        with tc.tile_pool(name="const", bufs=1) as const, \
             tc.tile_pool(name="sb", bufs=1) as sb, \
             tc.tile_pool(name="esp", bufs=1) as esp, \
             tc.tile_pool(name="normp", bufs=1) as normp, \
             tc.tile_pool(name="ps", bufs=1, space="PSUM") as ps:

            # psum rotation over 3 two-bank slots
            scn = [0]
            def sc_tile(shape, nm):
                t = ps.tile(shape, F32, name=nm, tag=f"sc{scn[0] % 3}")
                scn[0] += 1
                return t

            # ================= weights (once per core) =================
            def flat_w(src, nm):
                tiles = []
                for X in range(2):
                    stg = const.tile([128, 128], F32, name=f"stg_{nm}{X}",
                                     tag="wstg", bufs=2)
                    nc.vector.memset(stg[:], 0.0)
                    for g in range(4):
                        nc.sync.dma_start(stg[:, 32 * g:32 * g + K],
                                          w_d[src][4 * X + g])
                    t = const.tile([128, 128], BF16, name=f"{nm}{X}")
                    nc.vector.tensor_copy(t[:], stg[:])
                    tiles.append(t)
                return tiles

            WK = flat_w("W_key_custom", "wk")
            WKC = flat_w("W_key_charge", "wkc")
            WQ1 = flat_w("W_query_custom_1", "wq1")
            WQC1 = flat_w("W_query_charge_1", "wqc1")
            WQ2 = flat_w("W_query_custom", "wq2")

            WO = []
            for X in range(2):
                stg = const.tile([128, 128], F32, name=f"wostg{X}", tag="wstg", bufs=2)
                nc.vector.memset(stg[:], 0.0)
                for g in range(4):
                    nc.sync.dma_start(stg[32 * g:32 * g + K, :], wout_d[4 * X + g])
                t = const.tile([128, 128], BF16, name=f"wo{X}")
                nc.vector.tensor_copy(t[:], stg[:])
                WO.append(t)

            def val_w(wname, nm):
                stg = const.tile([128, 136], F32, name=f"stg_{nm}", tag="wstgv", bufs=2)
                nc.vector.memset(stg[:], 0.0)
                for h in range(H):
                    nc.sync.dma_start(stg[:, 17 * h:17 * h + K], w_d[wname][h])
                t = const.tile([128, 136], BF16, name=nm)
                nc.vector.tensor_copy(t[:], stg[:])
                return t

            WV = val_w("W_val_custom", "wv")
            WVC = val_w("W_val_charge", "wvc")

            state = {}

            # ---------------- Phase A: loads, values, projections (128x128)
            def phase_A(b):
                st = {}
                qTf = sb.tile([128, N], F32, name=f"qTf{b}", tag="qTf")
                nc.sync.dma_start(qTf[:], qT_d[b])
                hTf = sb.tile([128, N], F32, name=f"hTf{b}", tag="hTf")
                nc.sync.dma_start(hTf[:], hT_d[b])
                qTb = sb.tile([128, N], BF16, name=f"qTb{b}", tag="qTb")
                nc.vector.tensor_copy(qTb[:], qTf[:])
                hTb = sb.tile([128, N], BF16, name=f"hTb{b}", tag="hTb")
                nc.vector.tensor_copy(hTb[:], hTf[:])

                # values Vaug[j]: [128, 160] bf16 (136 data + 24 zero pad),
                # ones at col 17h+16
                va = []
                for j in range(8):
                    pv = sc_tile([128, 136], f"pv{b}_{j}")
                    nc.tensor.matmul(pv[:], hTb[:, 128 * j:128 * j + 128], WV[:],
                                     start=True, stop=True)
                    v = sb.tile([128, 160], BF16, name=f"va{b}_{j}", tag=f"va{j}")
                    nc.vector.tensor_copy(v[:, 0:136], pv[:])
                    nc.vector.memset(v[:, 136:160], 0.0)
                    v3 = v[:, 0:136].rearrange("p (h s) -> p h s", h=H)
                    nc.vector.memset(v3[:, :, K:K + 1], 1.0)
                    va.append(v)
                pvs = sc_tile([128, 136], f"pvs{b}")
                nc.tensor.matmul(pvs[0:S, :], hTb[:, 0:S], WVC[:],
                                 start=True, stop=True)
                vst4 = sb.tile([128, 160], BF16, name=f"vst4{b}", tag="vst4")
                nc.vector.tensor_copy(vst4[0:S, 0:136], pvs[0:S, :])
                nc.vector.memset(vst4[0:S, 136:160], 0.0)
                vst3 = vst4[0:S, 0:136].rearrange("p (h s) -> p h s", h=H)
                nc.vector.memset(vst3[:, :, K:K + 1], 1.0)
                for g in range(1, 4):
                    nc.vector.tensor_copy(vst4[32 * g:32 * g + S, :], vst4[0:S, :])

                kt, q1, q2 = [], [], []
                for X in range(2):
                    pk = sc_tile([128, N], f"pk{b}_{X}")
                    nc.tensor.matmul(pk[:, 0:S], WKC[X][:], hTb[:, 0:S],
                                     start=True, stop=True)
                    nc.tensor.matmul(pk[:, S:512], WK[X][:], hTb[:, S:512],
                                     start=True, stop=True)
                    nc.tensor.matmul(pk[:, 512:N], WK[X][:], hTb[:, 512:N],
                                     start=True, stop=True)
                    k_ = sb.tile([128, N], BF16, name=f"kt{b}_{X}", tag=f"kt{X}")
                    nc.vector.tensor_copy(k_[:], pk[:])
                    kt.append(k_)
                    p1 = sc_tile([128, N], f"p1{b}_{X}")
                    nc.tensor.matmul(p1[:, 0:S], WQC1[X][:], qTb[:, 0:S],
                                     start=True, stop=True)
                    nc.tensor.matmul(p1[:, S:512], WQ1[X][:], qTb[:, S:512],
                                     start=True, stop=True)
                    nc.tensor.matmul(p1[:, 512:N], WQ1[X][:], qTb[:, 512:N],
                                     start=True, stop=True)
                    q1_ = sb.tile([128, N], BF16, name=f"q1{b}_{X}", tag=f"q1{X}")
                    nc.vector.tensor_copy(q1_[:], p1[:])
                    q1.append(q1_)
                    p2 = sc_tile([128, N], f"p2{b}_{X}")
                    nc.tensor.matmul(p2[:, 0:512], WQ2[X][:], qTb[:, 0:512],
                                     start=True, stop=True)
                    nc.tensor.matmul(p2[:, 512:N], WQ2[X][:], qTb[:, 512:N],
                                     start=True, stop=True)
                    q2_ = sb.tile([128, N], BF16, name=f"q2{b}_{X}", tag=f"q2{X}")
                    nc.vector.tensor_copy(q2_[:], p2[:])
                    q2.append(q2_)
                st["kt"], st["q1"], st["q2"] = kt, q1, q2
                st["va"], st["vst4"] = va, vst4
                return st

            # ------- Phase B tile: tt-scores for (X, j) (32x128 row tiling)
            def phase_B_tile(b, st, X, j):
                es = st.setdefault("es", {})
                kt, q1 = st["kt"], st["q1"]
                for g in range(4):
                    h = 4 * X + g
                    stp = sc_tile([128, N], f"st{b}_{h}_{j}")
                    lhs = kt[X][32 * g:32 * g + K, 128 * j:128 * j + 128]
                    nc.tensor.matmul(stp[:, 0:512], lhs,
                                     q1[X][32 * g:32 * g + K, 0:512],
                                     start=True, stop=True,
                                     tile_position=(32 * g, 0))
                    nc.tensor.matmul(stp[:, 512:N], lhs,
                                     q1[X][32 * g:32 * g + K, 512:N],
                                     start=True, stop=True,
                                     tile_position=(32 * g, 0))
                    e_ = esp.tile([128, N], BF16, name=f"es{b}_{h}_{j}",
                                  tag=f"es{h}_{j}")
                    nc.scalar.activation(e_[:], stp[:], EXP, scale=NORM)
                    if j == 0:
                        nc.vector.memset(e_[0:S, :], 0.0)
                    es[(h, j)] = e_

            # ------- Phase TS(X): ts-scores + station AV + s-side norm prep
            def phase_TS(b, st, X):
                kt, q2, vst4 = st["kt"], st["q2"], st["vst4"]
                ps2 = sc_tile([128, N], f"ps2{b}_{X}")
                for g in range(4):
                    lhs2 = kt[X][32 * g:32 * g + K, 0:S]
                    for half in range(2):
                        nc.tensor.matmul(
                            ps2[32 * g:32 * g + S, 512 * half:512 * half + 512],
                            lhs2,
                            q2[X][32 * g:32 * g + K, 512 * half:512 * half + 512],
                            start=True, stop=True,
                            tile_position=(32 * g, 32 * g))
                e2 = sb.tile([128, N], BF16, name=f"es2{b}_{X}", tag="es2")
                nc.scalar.activation(e2[:], ps2[:], EXP, scale=NORM)
                rp = sb.tile([128, N], F32, name=f"rawpts{b}_{X}", tag=f"rpts{X}")
                for half in range(2):
                    pts = ps.tile([128, 512], F32, name=f"pts{b}_{X}_{half}",
                                  tag="pav", bufs=2)
                    for g in range(4):
                        h = 4 * X + g
                        nc.tensor.matmul(
                            pts[32 * g:32 * g + 32, :],
                            vst4[32 * g:32 * g + S, 17 * h:17 * h + 32],
                            e2[32 * g:32 * g + S, 512 * half:512 * half + 512],
                            start=True, stop=True,
                            tile_position=(32 * g, 32 * g))
                    nc.vector.tensor_copy(rp[:, 512 * half:512 * half + 512],
                                          pts[:])
                st.setdefault("rawpts", {})[X] = rp
                # s-side denominators -> rbs bands -> reciprocal
                rbs = sb.tile([128, N], F32, name=f"rbs{b}_{X}", tag="rbs")
                if b == 0 and X == 0:
                    nc.vector.memset(rbs[:], 1.0)
                for g in range(4):
                    srow = normp.tile([1, N], F32, name=f"srs{b}_{X}_{g}",
                                      tag="srow", bufs=2)
                    nc.sync.dma_start(srow[:], rp[32 * g + K:32 * g + 17, :])
                    rb16 = normp.tile([16, N], F32, name=f"rb16s{b}_{X}_{g}",
                                      tag="rb16")
                    nc.gpsimd.partition_broadcast(rb16[:], srow[:])
                    nc.sync.dma_start(rbs[32 * g:32 * g + K, :], rb16[:])
                nc.vector.reciprocal_approx_fast(rbs[:], rbs[:])
                st.setdefault("rbs", {})[X] = rbs

            # ------- Phase C (AV, 128x32 col tiling), j-sliced
            def phase_C_j(b, st, X, j):
                va, es = st["va"], st["es"]
                if j == 0:
                    st[f"pav{X}"] = [
                        ps.tile([128, 512], F32, name=f"pav{b}_{X}_{half}",
                                tag="pav", bufs=2)
                        for half in range(2)]
                pav = st[f"pav{X}"]
                for half in range(2):
                    for g in range(4):
                        h = 4 * X + g
                        nc.tensor.matmul(
                            pav[half][32 * g:32 * g + 32, :],
                            va[j][:, 17 * h:17 * h + 32],
                            es[(h, j)][:, 512 * half:512 * half + 512],
                            start=(j == 0), stop=(j == 7),
                            tile_position=(0, 32 * g),
                            skip_group_check=True)

            def phase_C_fin(b, st, X):
                pav = st.pop(f"pav{X}")
                ra = sb.tile([128, N], F32, name=f"rawav{b}_{X}", tag=f"rav{X}")
                for half in range(2):
                    nc.vector.tensor_copy(ra[:, 512 * half:512 * half + 512],
                                          pav[half][:])
                st.setdefault("rawav", {})[X] = ra

            # ------- Phase NT(X): t-side norm + heads assembly (no PE)
            def phase_NT(b, st, X):
                ra, rp = st["rawav"][X], st["rawpts"][X]
                rbs = st["rbs"][X]
                rbt = sb.tile([128, N], F32, name=f"rbt{b}_{X}", tag="rbt")
                if b == 0 and X == 0:
                    nc.vector.memset(rbt[:], 1.0)
                for g in range(4):
                    srow = normp.tile([1, N], F32, name=f"srt{b}_{X}_{g}",
                                      tag="srow", bufs=2)
                    nc.sync.dma_start(srow[:], ra[32 * g + K:32 * g + 17, :])
                    rb16 = normp.tile([16, N], F32, name=f"rb16t{b}_{X}_{g}",
                                      tag="rb16")
                    nc.gpsimd.partition_broadcast(rb16[:], srow[:])
                    nc.sync.dma_start(rbt[32 * g:32 * g + K, :], rb16[:])
                nc.vector.reciprocal_approx_fast(rbt[:], rbt[:])
                tmp = sb.tile([128, N], BF16, name=f"tmp{b}_{X}", tag="tmp")
                hx = sb.tile([128, N], BF16, name=f"heads{b}_{X}",
                             tag=f"heads{X}")
                nc.vector.tensor_mul(hx[:], ra[:], rbt[:])
                nc.vector.tensor_mul(tmp[:, S:N], rp[:, S:N], rbs[:, S:N])
                nc.vector.tensor_add(hx[:, S:N], hx[:, S:N], tmp[:, S:N])
                st.setdefault("heads", {})[X] = hx

            # ---------------- Phase D: output projection (128x128)
            def phase_D(b, st):
                heads = st["heads"]
                for nt in range(8):
                    po = sc_tile([128, 128], f"po{b}_{nt}")
                    nc.tensor.matmul(po[:], heads[0][:, 128 * nt:128 * nt + 128],
                                     WO[0][:], start=True, stop=False,
                                     skip_group_check=True)
                    nc.tensor.matmul(po[:], heads[1][:, 128 * nt:128 * nt + 128],
                                     WO[1][:], start=False, stop=True,
                                     skip_group_check=True)
                    ot = normp.tile([128, 128], F32, name=f"ot{b}_{nt}", tag="ot")
                    nc.vector.tensor_copy(ot[:], po[:])
                    nc.sync.dma_start(out_d[b, 128 * nt:128 * nt + 128, :], ot[:])

            # ---------------- software-pipelined emission
            # PE stream: scores(X0 j0..j7), [scores(X1 j) + AV(X0 j)] x8,
            # then AV(X1 j) with D(b-1)/A(b+1)/TS(b+1) as filler. Scalar queue
            # stays continuous: es2(b), es(b) x64, es2(b+1), ...
            states = {}
            states[0] = phase_A(0)
            phase_TS(0, states[0], 0)
            phase_TS(0, states[0], 1)
            for b in range(BPC):
                st = states[b]
                for j in range(8):
                    phase_B_tile(b, st, 0, j)
                for j in range(8):
                    phase_B_tile(b, st, 1, j)
                    phase_C_j(b, st, 0, j)
                phase_C_fin(b, st, 0)
                if b > 0:
                    phase_D(b - 1, states[b - 1])
                    del states[b - 1]
                phase_NT(b, st, 0)
                if b + 1 < BPC:
                    states[b + 1] = phase_A(b + 1)
                for j in range(8):
                    phase_C_j(b, st, 1, j)
                phase_C_fin(b, st, 1)
                phase_NT(b, st, 1)
                if b + 1 < BPC:
                    phase_TS(b + 1, states[b + 1], 0)
                    phase_TS(b + 1, states[b + 1], 1)
            phase_D(BPC - 1, states[BPC - 1])

    nc.compile()
    return nc


def _get_nc():
    if "nc" not in _CACHE:
        _CACHE["nc"] = _build()
    return _CACHE["nc"]


def _kernel_jax(q, h, Ws):
    """Batch-sharded (data-parallel) attention on the 8 NeuronCores via pmap."""
    import jax, jax.numpy as jnp
    S_ = S
    NORMc = np.float32(NORM)

    def one_shard(q, h, W_query_custom, W_query_custom_1, W_key_custom,
                  W_val_custom, W_query_charge_1, W_key_charge, W_val_charge,
                  W_out):
        h_st, h_tk = h[:, :S_], h[:, S_:]
        q_st, q_tk = q[:, :S_], q[:, S_:]
        proj = lambda x, W: jnp.einsum('bnd,hdk->hbnk', x, W)
        K_c = proj(h_tk, W_key_custom)
        V_c = proj(h_tk, W_val_custom)
        K_s = proj(h_st, W_key_charge)
        V_s = proj(h_st, W_val_charge)
        Q_tt = proj(q_tk, W_query_custom_1)
        A_tt = jax.nn.softmax(NORMc * jnp.einsum('hbqk,hbtk->hbqt', Q_tt, K_c), axis=-1)
        heads_t = jnp.einsum('hbqt,hbtk->hbqk', A_tt, V_c)
        Q_ts = proj(q_tk, W_query_custom)
        A_ts = jax.nn.softmax(NORMc * jnp.einsum('hbqk,hbsk->hbqs', Q_ts, K_s), axis=-1)
        heads_t = heads_t + jnp.einsum('hbqs,hbsk->hbqk', A_ts, V_s)
        Q_st = proj(q_st, W_query_charge_1)
        A_st = jax.nn.softmax(NORMc * jnp.einsum('hbqk,hbtk->hbqt', Q_st, K_c), axis=-1)
        heads_s = jnp.einsum('hbqt,hbtk->hbqk', A_st, V_c)
        heads = jnp.concatenate([heads_s, heads_t], axis=2)
        return jnp.einsum('hbnk,hke->bne', heads, W_out)

    if "pmap_fn" not in _CACHE:
        _CACHE["pmap_fn"] = jax.pmap(one_shard, axis_name="i")
    f = _CACHE["pmap_fn"]
    qs = q.reshape(NCORES, BPC, N, D)
    hs = h.reshape(NCORES, BPC, N, D)
    wkey = tuple(w.tobytes()[:64] for w in Ws)
    if _CACHE.get("wkey") != wkey:
        _CACHE["wrep"] = [jax.device_put_replicated(jnp.asarray(w), jax.devices()[:NCORES])
                          for w in Ws]
        _CACHE["wkey"] = wkey
    out = f(qs, hs, *_CACHE["wrep"])
    return np.asarray(out).reshape(B, N, E)


USE_BASS = True


def kernel(q, h, W_query_custom, W_query_custom_1, W_key_custom, W_val_custom,
           W_query_charge_1, W_key_charge, W_val_charge, W_out, _trace=False):
    if not USE_BASS:
        Ws = [np.asarray(w, np.float32) for w in
              (W_query_custom, W_query_custom_1, W_key_custom, W_val_custom,
               W_query_charge_1, W_key_charge, W_val_charge, W_out)]
        return _kernel_jax(np.asarray(q, np.float32), np.asarray(h, np.float32), Ws)
    return _kernel_bass(q, h, W_query_custom, W_query_custom_1, W_key_custom,
                        W_val_custom, W_query_charge_1, W_key_charge,
                        W_val_charge, W_out, _trace)


def _kernel_bass(q, h, W_query_custom, W_query_custom_1, W_key_custom, W_val_custom,
                 W_query_charge_1, W_key_charge, W_val_charge, W_out, _trace=False):
    from concourse.bass_utils import run_bass_kernel_spmd

    nc = _get_nc()
    qT = np.ascontiguousarray(np.asarray(q, dtype=np.float32).transpose(0, 2, 1))
    hT = np.ascontiguousarray(np.asarray(h, dtype=np.float32).transpose(0, 2, 1))
    ws = {
        "W_query_custom": W_query_custom, "W_query_custom_1": W_query_custom_1,
        "W_key_custom": W_key_custom, "W_val_custom": W_val_custom,
        "W_query_charge_1": W_query_charge_1, "W_key_charge": W_key_charge,
        "W_val_charge": W_val_charge, "W_out": W_out,
    }
    ws = {k: np.ascontiguousarray(np.asarray(v, dtype=np.float32))
          for k, v in ws.items()}
    in_maps = []
    for c in range(NCORES):
        m = {"qT": qT[c * BPC:(c + 1) * BPC], "hT": hT[c * BPC:(c + 1) * BPC]}
        m.update(ws)
        in_maps.append(m)
    res = run_bass_kernel_spmd(nc, in_maps, core_ids=list(range(NCORES)),
                               trace=_trace)
    out = np.concatenate([res.results[c]["out"] for c in range(NCORES)], axis=0)
    if _trace:
        _CACHE["last_results"] = res
    return out


# revision 21
# speedup vs baseline: 1.0914x; 1.0291x over previous
"""Trainium2 Bass kernel for nn_HMHA (heterogeneous multi-head attention).

Reference semantics (B=32, N=1024, D=128, H=8, K=16, S=21 stations, T=1003 tasks):
  - 7 per-head projections of q/h slices, three attention blocks
    (task->task, task->station, station->task), all softmaxed over keys,
    combined and projected by W_out.

Sharding: data-parallel over batch across 8 cores (4 batches/core).

v2 layout (per core, per batch), all PE inputs bf16:
  - Heads split into two buffers: A = heads 0-3, B = heads 4-7, head g of a
    buffer at 32-aligned partition band 32g (PE tile_position row/col bands).
  - Flat projections: kt/q1/q2 [128(4hx32band), N] via single [128,128] flat
    weight matmuls (cols 0:21 use the charge/station weights, 21: the task
    weights, matching position-dependent projection in the reference).
  - tt-scores: row-tiled (32x128 mode) matmuls, 2 heads concurrent,
    psum [128keys, N] -> exp (scalar ACT, scale 0.25) -> es bf16 SBUF.
  - ts-scores + station AV: 32x32 diagonal tiles, 4 heads packed per psum.
  - AV: col-tiled (128x32 mode), 4 heads packed into [128, 512] psum halves,
    Vaug carries a ones-slot per head -> row 32g+16 = softmax denominator.
  - Normalization: denominators DMAd to [1,N] tiles, reciprocal on DVE,
    gpsimd partition_broadcast + DVE band copy, all-SBUF band-aligned muls.
  - Output: heads [128(8hx16), N] bf16 x flat W_out [128, 128] accumulated
    over the two buffers per 128-col n-tile.
"""
import numpy as np

NUM_STATION = 20
S = NUM_STATION + 1          # 21
H = 8
D = 128
K = 16
E = 128
N = 1024
B = 32
NCORES = 8
BPC = B // NCORES            # 4 batches per core
NORM = 0.25                  # 1/sqrt(16)

_CACHE = {}


def _build():
    import concourse.bass as bass
    import concourse.tile as tile
    from concourse import bacc, mybir

    F32 = mybir.dt.float32
    BF16 = mybir.dt.bfloat16
    EXP = mybir.ActivationFunctionType.Exp

    nc = bacc.Bacc("TRN2", target_bir_lowering=False, debug=False,
                   num_devices=NCORES)

    qT_d = nc.dram_tensor("qT", [BPC, D, N], F32, kind="ExternalInput").ap()
    hT_d = nc.dram_tensor("hT", [BPC, D, N], F32, kind="ExternalInput").ap()
    wnames = ["W_query_custom", "W_query_custom_1", "W_key_custom",
              "W_val_custom", "W_query_charge_1", "W_key_charge",
              "W_val_charge"]
    w_d = {n: nc.dram_tensor(n, [H, D, K], F32, kind="ExternalInput").ap()
           for n in wnames}
    wout_d = nc.dram_tensor("W_out", [H, K, E], F32, kind="ExternalInput").ap()
    out_d = nc.dram_tensor("out", [BPC, N, E], F32, kind="ExternalOutput").ap()

    with tile.TileContext(nc) as tc:
        with tc.tile_pool(name="const", bufs=1) as const, \
             tc.tile_pool(name="sb", bufs=1) as sb, \
             tc.tile_pool(name="esp", bufs=1) as esp, \
             tc.tile_pool(name="normp", bufs=1) as normp, \
             tc.tile_pool(name="ps", bufs=1, space="PSUM") as ps:

            # psum rotation over 3 two-bank slots
            scn = [0]
            def sc_tile(shape, nm):
                t = ps.tile(shape, F32, name=nm, tag=f"sc{scn[0] % 3}")
                scn[0] += 1
                return t

            # ================= weights (once per core) =================
            def flat_w(src, nm):
                tiles = []
                for X in range(2):
                    stg = const.tile([128, 128], F32, name=f"stg_{nm}{X}",
                                     tag="wstg", bufs=2)
                    nc.vector.memset(stg[:], 0.0)
                    for g in range(4):
                        nc.sync.dma_start(stg[:, 32 * g:32 * g + K],
                                          w_d[src][4 * X + g])
                    t = const.tile([128, 128], BF16, name=f"{nm}{X}")
                    nc.vector.tensor_copy(t[:], stg[:])
                    tiles.append(t)
                return tiles

            WK = flat_w("W_key_custom", "wk")
            WKC = flat_w("W_key_charge", "wkc")
            WQ1 = flat_w("W_query_custom_1", "wq1")
            WQC1 = flat_w("W_query_charge_1", "wqc1")
            WQ2 = flat_w("W_query_custom", "wq2")

            WO = []
            for X in range(2):
                stg = const.tile([128, 128], F32, name=f"wostg{X}", tag="wstg", bufs=2)
                nc.vector.memset(stg[:], 0.0)
                for g in range(4):
                    nc.sync.dma_start(stg[32 * g:32 * g + K, :], wout_d[4 * X + g])
                t = const.tile([128, 128], BF16, name=f"wo{X}")
                nc.vector.tensor_copy(t[:], stg[:])
                WO.append(t)

            def val_w(wname, nm):
                stg = const.tile([128, 136], F32, name=f"stg_{nm}", tag="wstgv", bufs=2)
                nc.vector.memset(stg[:], 0.0)
                for h in range(H):
                    nc.sync.dma_start(stg[:, 17 * h:17 * h + K], w_d[wname][h])
                t = const.tile([128, 136], BF16, name=nm)
                nc.vector.tensor_copy(t[:], stg[:])
                return t

            WV = val_w("W_val_custom", "wv")
            WVC = val_w("W_val_charge", "wvc")

            state = {}

            # ---------------- Phase A: loads, values, projections (128x128)
            def phase_A(b):
                st = {}
                qTf = sb.tile([128, N], F32, name=f"qTf{b}", tag="qTf")
                nc.sync.dma_start(qTf[:], qT_d[b])
                hTf = sb.tile([128, N], F32, name=f"hTf{b}", tag="qTf")
                nc.sync.dma_start(hTf[:], hT_d[b])
                qTb = sb.tile([128, N], BF16, name=f"qTb{b}", tag="qTb")
                nc.vector.tensor_copy(qTb[:], qTf[:])
                hTb = sb.tile([128, N], BF16, name=f"hTb{b}", tag="hTb")
                nc.vector.tensor_copy(hTb[:], hTf[:])

                # values Vaug[j]: [128, 160] bf16 (136 data + 24 zero pad),
                # ones at col 17h+16
                va = []
                for j in range(8):
                    pv = sc_tile([128, 136], f"pv{b}_{j}")
                    nc.tensor.matmul(pv[:], hTb[:, 128 * j:128 * j + 128], WV[:],
                                     start=True, stop=True)
                    v = sb.tile([128, 160], BF16, name=f"va{b}_{j}", tag=f"va{j}", bufs=2)
                    nc.vector.tensor_copy(v[:, 0:136], pv[:])
                    nc.vector.memset(v[:, 136:160], 0.0)
                    v3 = v[:, 0:136].rearrange("p (h s) -> p h s", h=H)
                    nc.vector.memset(v3[:, :, K:K + 1], 1.0)
                    va.append(v)
                pvs = sc_tile([128, 136], f"pvs{b}")
                nc.tensor.matmul(pvs[0:S, :], hTb[:, 0:S], WVC[:],
                                 start=True, stop=True)
                vst4 = sb.tile([128, 160], BF16, name=f"vst4{b}", tag="vst4", bufs=2)
                nc.vector.tensor_copy(vst4[0:S, 0:136], pvs[0:S, :])
                nc.vector.memset(vst4[0:S, 136:160], 0.0)
                vst3 = vst4[0:S, 0:136].rearrange("p (h s) -> p h s", h=H)
                nc.vector.memset(vst3[:, :, K:K + 1], 1.0)
                for g in range(1, 4):
                    nc.vector.tensor_copy(vst4[32 * g:32 * g + S, :], vst4[0:S, :])

                kt, q1, q2 = [], [], []
                for X in range(2):
                    pk = sc_tile([128, N], f"pk{b}_{X}")
                    nc.tensor.matmul(pk[:, 0:S], WKC[X][:], hTb[:, 0:S],
                                     start=True, stop=True)
                    nc.tensor.matmul(pk[:, S:512], WK[X][:], hTb[:, S:512],
                                     start=True, stop=True)
                    nc.tensor.matmul(pk[:, 512:N], WK[X][:], hTb[:, 512:N],
                                     start=True, stop=True)
                    k_ = sb.tile([128, N], BF16, name=f"kt{b}_{X}", tag=f"kt{X}")
                    nc.vector.tensor_copy(k_[:], pk[:])
                    kt.append(k_)
                    p1 = sc_tile([128, N], f"p1{b}_{X}")
                    nc.tensor.matmul(p1[:, 0:S], WQC1[X][:], qTb[:, 0:S],
                                     start=True, stop=True)
                    nc.tensor.matmul(p1[:, S:512], WQ1[X][:], qTb[:, S:512],
                                     start=True, stop=True)
                    nc.tensor.matmul(p1[:, 512:N], WQ1[X][:], qTb[:, 512:N],
                                     start=True, stop=True)
                    q1_ = sb.tile([128, N], BF16, name=f"q1{b}_{X}", tag=f"q1{X}")
                    nc.vector.tensor_copy(q1_[:], p1[:])
                    q1.append(q1_)
                    p2 = sc_tile([128, N], f"p2{b}_{X}")
                    nc.tensor.matmul(p2[:, 0:512], WQ2[X][:], qTb[:, 0:512],
                                     start=True, stop=True)
                    nc.tensor.matmul(p2[:, 512:N], WQ2[X][:], qTb[:, 512:N],
                                     start=True, stop=True)
                    q2_ = sb.tile([128, N], BF16, name=f"q2{b}_{X}", tag=f"q2{X}")
                    nc.vector.tensor_copy(q2_[:], p2[:])
                    q2.append(q2_)
                st["kt"], st["q1"], st["q2"] = kt, q1, q2
                st["va"], st["vst4"] = va, vst4
                return st

            # ------- Phase B tile: tt-scores for (X, j) (32x128 row tiling)
            def phase_B_tile(b, st, X, j):
                es = st.setdefault("es", {})
                kt, q1 = st["kt"], st["q1"]
                for g in range(4):
                    h = 4 * X + g
                    stp = sc_tile([128, N], f"st{b}_{h}_{j}")
                    lhs = kt[X][32 * g:32 * g + K, 128 * j:128 * j + 128]
                    nc.tensor.matmul(stp[:, 0:512], lhs,
                                     q1[X][32 * g:32 * g + K, 0:512],
                                     start=True, stop=True,
                                     tile_position=(32 * g, 0))
                    nc.tensor.matmul(stp[:, 512:N], lhs,
                                     q1[X][32 * g:32 * g + K, 512:N],
                                     start=True, stop=True,
                                     tile_position=(32 * g, 0))
                    e_ = esp.tile([128, N], BF16, name=f"es{b}_{h}_{j}",
                                  tag=f"es{h}_{j}")
                    nc.scalar.activation(e_[:], stp[:], EXP, scale=NORM)
                    if j == 0:
                        nc.vector.memset(e_[0:S, :], 0.0)
                    es[(h, j)] = e_

            # ------- Phase TS(X): ts-scores + station AV + s-side norm prep
            def phase_TS(b, st, X):
                kt, q2, vst4 = st["kt"], st["q2"], st["vst4"]
                ps2 = sc_tile([128, N], f"ps2{b}_{X}")
                for g in range(4):
                    lhs2 = kt[X][32 * g:32 * g + K, 0:S]
                    for half in range(2):
                        nc.tensor.matmul(
                            ps2[32 * g:32 * g + S, 512 * half:512 * half + 512],
                            lhs2,
                            q2[X][32 * g:32 * g + K, 512 * half:512 * half + 512],
                            start=True, stop=True,
                            tile_position=(32 * g, 32 * g))
                e2 = sb.tile([128, N], BF16, name=f"es2{b}_{X}", tag="es2")
                nc.scalar.activation(e2[:], ps2[:], EXP, scale=NORM)
                rp = sb.tile([128, N], F32, name=f"rawpts{b}_{X}", tag=f"rpts{X}")
                for half in range(2):
                    pts = ps.tile([128, 512], F32, name=f"pts{b}_{X}_{half}",
                                  tag="pav", bufs=2)
                    for g in range(4):
                        h = 4 * X + g
                        nc.tensor.matmul(
                            pts[32 * g:32 * g + 32, :],
                            vst4[32 * g:32 * g + S, 17 * h:17 * h + 32],
                            e2[32 * g:32 * g + S, 512 * half:512 * half + 512],
                            start=True, stop=True,
                            tile_position=(32 * g, 32 * g))
                    nc.vector.tensor_copy(rp[:, 512 * half:512 * half + 512],
                                          pts[:])
                st.setdefault("rawpts", {})[X] = rp
                # s-side denominators -> rbs bands -> reciprocal
                rbs = sb.tile([128, N], F32, name=f"rbs{b}_{X}", tag=f"rbs{X}")
                nc.vector.memset(rbs[:], 1.0)
                for g in range(4):
                    srow = normp.tile([1, N], F32, name=f"srs{b}_{X}_{g}",
                                      tag="srow", bufs=2)
                    nc.sync.dma_start(srow[:], rp[32 * g + K:32 * g + 17, :])
                    rb16 = normp.tile([16, N], F32, name=f"rb16s{b}_{X}_{g}",
                                      tag="rb16")
                    nc.gpsimd.partition_broadcast(rb16[:], srow[:])
                    nc.vector.tensor_copy(rbs[32 * g:32 * g + K, :], rb16[:])
                nc.vector.reciprocal_approx_fast(rbs[:], rbs[:])
                st.setdefault("rbs", {})[X] = rbs

            # ------- Phase C (AV, 128x32 col tiling), j-sliced
            def phase_C_j(b, st, X, j):
                va, es = st["va"], st["es"]
                if j == 0:
                    st[f"pav{X}"] = [
                        ps.tile([128, 512], F32, name=f"pav{b}_{X}_{half}",
                                tag="pav", bufs=2)
                        for half in range(2)]
                pav = st[f"pav{X}"]
                for half in range(2):
                    for g in range(4):
                        h = 4 * X + g
                        nc.tensor.matmul(
                            pav[half][32 * g:32 * g + 32, :],
                            va[j][:, 17 * h:17 * h + 32],
                            es[(h, j)][:, 512 * half:512 * half + 512],
                            start=(j == 0), stop=(j == 7),
                            tile_position=(0, 32 * g),
                            skip_group_check=True)

            def phase_C_fin(b, st, X):
                pav = st.pop(f"pav{X}")
                ra = sb.tile([128, N], F32, name=f"rawav{b}_{X}", tag=f"rav{X}")
                for half in range(2):
                    nc.vector.tensor_copy(ra[:, 512 * half:512 * half + 512],
                                          pav[half][:])
                st.setdefault("rawav", {})[X] = ra

            # ------- Phase NT(X): t-side norm + heads assembly (no PE)
            def phase_NT(b, st, X):
                ra, rp = st["rawav"][X], st["rawpts"][X]
                rbs = st["rbs"][X]
                rbt = sb.tile([128, N], F32, name=f"rbt{b}_{X}", tag="rbt")
                if b == 0 and X == 0:
                    nc.vector.memset(rbt[:], 1.0)
                for g in range(4):
                    srow = normp.tile([1, N], F32, name=f"srt{b}_{X}_{g}",
                                      tag="srow", bufs=2)
                    nc.sync.dma_start(srow[:], ra[32 * g + K:32 * g + 17, :])
                    rb16 = normp.tile([16, N], F32, name=f"rb16t{b}_{X}_{g}",
                                      tag="rb16")
                    nc.gpsimd.partition_broadcast(rb16[:], srow[:])
                    nc.vector.tensor_copy(rbt[32 * g:32 * g + K, :], rb16[:])
                nc.vector.reciprocal_approx_fast(rbt[:], rbt[:])
                hx = sb.tile([128, N], BF16, name=f"heads{b}_{X}",
                             tag=f"heads{X}")
                nc.vector.tensor_mul(hx[:], ra[:], rbt[:])
                nc.vector.tensor_mul(rbs[:, S:N], rp[:, S:N], rbs[:, S:N])
                nc.vector.tensor_add(hx[:, S:N], hx[:, S:N], rbs[:, S:N])
                st.setdefault("heads", {})[X] = hx

            # ---------------- Phase D: output projection (128x128)
            def phase_D(b, st):
                heads = st["heads"]
                for nt in range(8):
                    po = sc_tile([128, 128], f"po{b}_{nt}")
                    nc.tensor.matmul(po[:], heads[0][:, 128 * nt:128 * nt + 128],
                                     WO[0][:], start=True, stop=False,
                                     skip_group_check=True)
                    nc.tensor.matmul(po[:], heads[1][:, 128 * nt:128 * nt + 128],
                                     WO[1][:], start=False, stop=True,
                                     skip_group_check=True)
                    ot = normp.tile([128, 128], F32, name=f"ot{b}_{nt}", tag="ot")
                    nc.vector.tensor_copy(ot[:], po[:])
                    nc.sync.dma_start(out_d[b, 128 * nt:128 * nt + 128, :], ot[:])

            # ---------------- software-pipelined emission
            # Contiguous PE mode runs per cycle:
            #   B(X0)+B(X1) [32x128] -> D(b-1)+A(b+1) [128x128] ->
            #   C(X0)/C(X1) [128x32] -> TS(b+1) [32x32]
            states = {}
            states[0] = phase_A(0)
            phase_TS(0, states[0], 0)
            phase_TS(0, states[0], 1)
            for b in range(BPC):
                st = states[b]
                for j in range(8):
                    phase_B_tile(b, st, 0, j)
                for j in range(8):
                    phase_B_tile(b, st, 1, j)
                if b > 0:
                    phase_D(b - 1, states[b - 1])
                    del states[b - 1]
                if b + 1 < BPC:
                    states[b + 1] = phase_A(b + 1)
                for j in range(8):
                    phase_C_j(b, st, 0, j)
                phase_C_fin(b, st, 0)
                phase_NT(b, st, 0)
                for j in range(8):
                    phase_C_j(b, st, 1, j)
                phase_C_fin(b, st, 1)
                phase_NT(b, st, 1)
                if b + 1 < BPC:
                    phase_TS(b + 1, states[b + 1], 0)
                    phase_TS(b + 1, states[b + 1], 1)
            phase_D(BPC - 1, states[BPC - 1])

    nc.compile()
    return nc


def _get_nc():
    if "nc" not in _CACHE:
        _CACHE["nc"] = _build()
    return _CACHE["nc"]


def _kernel_jax(q, h, Ws):
    """Batch-sharded (data-parallel) attention on the 8 NeuronCores via pmap."""
    import jax, jax.numpy as jnp
    S_ = S
    NORMc = np.float32(NORM)

    def one_shard(q, h, W_query_custom, W_query_custom_1, W_key_custom,
                  W_val_custom, W_query_charge_1, W_key_charge, W_val_charge,
                  W_out):
        h_st, h_tk = h[:, :S_], h[:, S_:]
        q_st, q_tk = q[:, :S_], q[:, S_:]
        proj = lambda x, W: jnp.einsum('bnd,hdk->hbnk', x, W)
        K_c = proj(h_tk, W_key_custom)
        V_c = proj(h_tk, W_val_custom)
        K_s = proj(h_st, W_key_charge)
        V_s = proj(h_st, W_val_charge)
        Q_tt = proj(q_tk, W_query_custom_1)
        A_tt = jax.nn.softmax(NORMc * jnp.einsum('hbqk,hbtk->hbqt', Q_tt, K_c), axis=-1)
        heads_t = jnp.einsum('hbqt,hbtk->hbqk', A_tt, V_c)
        Q_ts = proj(q_tk, W_query_custom)
        A_ts = jax.nn.softmax(NORMc * jnp.einsum('hbqk,hbsk->hbqs', Q_ts, K_s), axis=-1)
        heads_t = heads_t + jnp.einsum('hbqs,hbsk->hbqk', A_ts, V_s)
        Q_st = proj(q_st, W_query_charge_1)
        A_st = jax.nn.softmax(NORMc * jnp.einsum('hbqk,hbtk->hbqt', Q_st, K_c), axis=-1)
        heads_s = jnp.einsum('hbqt,hbtk->hbqk', A_st, V_c)
        heads = jnp.concatenate([heads_s, heads_t], axis=2)
        return jnp.einsum('hbnk,hke->bne', heads, W_out)

    if "pmap_fn" not in _CACHE:
        _CACHE["pmap_fn"] = jax.pmap(one_shard, axis_name="i")
    f = _CACHE["pmap_fn"]
    qs = q.reshape(NCORES, BPC, N, D)
    hs = h.reshape(NCORES, BPC, N, D)
    wkey = tuple(w.tobytes()[:64] for w in Ws)
    if _CACHE.get("wkey") != wkey:
        _CACHE["wrep"] = [jax.device_put_replicated(jnp.asarray(w), jax.devices()[:NCORES])
                          for w in Ws]
        _CACHE["wkey"] = wkey
    out = f(qs, hs, *_CACHE["wrep"])
    return np.asarray(out).reshape(B, N, E)


USE_BASS = True


def kernel(q, h, W_query_custom, W_query_custom_1, W_key_custom, W_val_custom,
           W_query_charge_1, W_key_charge, W_val_charge, W_out, _trace=False):
    if not USE_BASS:
        Ws = [np.asarray(w, np.float32) for w in
              (W_query_custom, W_query_custom_1, W_key_custom, W_val_custom,
               W_query_charge_1, W_key_charge, W_val_charge, W_out)]
        return _kernel_jax(np.asarray(q, np.float32), np.asarray(h, np.float32), Ws)
    return _kernel_bass(q, h, W_query_custom, W_query_custom_1, W_key_custom,
                        W_val_custom, W_query_charge_1, W_key_charge,
                        W_val_charge, W_out, _trace)


def _kernel_bass(q, h, W_query_custom, W_query_custom_1, W_key_custom, W_val_custom,
                 W_query_charge_1, W_key_charge, W_val_charge, W_out, _trace=False):
    from concourse.bass_utils import run_bass_kernel_spmd

    nc = _get_nc()
    qT = np.ascontiguousarray(np.asarray(q, dtype=np.float32).transpose(0, 2, 1))
    hT = np.ascontiguousarray(np.asarray(h, dtype=np.float32).transpose(0, 2, 1))
    ws = {
        "W_query_custom": W_query_custom, "W_query_custom_1": W_query_custom_1,
        "W_key_custom": W_key_custom, "W_val_custom": W_val_custom,
        "W_query_charge_1": W_query_charge_1, "W_key_charge": W_key_charge,
        "W_val_charge": W_val_charge, "W_out": W_out,
    }
    ws = {k: np.ascontiguousarray(np.asarray(v, dtype=np.float32))
          for k, v in ws.items()}
    in_maps = []
    for c in range(NCORES):
        m = {"qT": qT[c * BPC:(c + 1) * BPC], "hT": hT[c * BPC:(c + 1) * BPC]}
        m.update(ws)
        in_maps.append(m)
    res = run_bass_kernel_spmd(nc, in_maps, core_ids=list(range(NCORES)),
                               trace=_trace)
    out = np.concatenate([res.results[c]["out"] for c in range(NCORES)], axis=0)
    if _trace:
        _CACHE["last_results"] = res
    return out


# revision 22
# speedup vs baseline: 1.4097x; 1.2917x over previous
"""Trainium2 Bass kernel for nn_HMHA (heterogeneous multi-head attention).

Reference semantics (B=32, N=1024, D=128, H=8, K=16, S=21 stations, T=1003 tasks):
  - 7 per-head projections of q/h slices, three attention blocks
    (task->task, task->station, station->task), all softmaxed over keys,
    combined and projected by W_out.

Sharding: data-parallel over batch across 8 cores (4 batches/core).

v2 layout (per core, per batch), all PE inputs bf16:
  - Heads split into two buffers: A = heads 0-3, B = heads 4-7, head g of a
    buffer at 32-aligned partition band 32g (PE tile_position row/col bands).
  - Flat projections: kt/q1/q2 [128(4hx32band), N] via single [128,128] flat
    weight matmuls (cols 0:21 use the charge/station weights, 21: the task
    weights, matching position-dependent projection in the reference).
  - tt-scores: row-tiled (32x128 mode) matmuls, 2 heads concurrent,
    psum [128keys, N] -> exp (scalar ACT, scale 0.25) -> es bf16 SBUF.
  - ts-scores + station AV: 32x32 diagonal tiles, 4 heads packed per psum.
  - AV: col-tiled (128x32 mode), 4 heads packed into [128, 512] psum halves,
    Vaug carries a ones-slot per head -> row 32g+16 = softmax denominator.
  - Normalization: denominators DMAd to [1,N] tiles, reciprocal on DVE,
    gpsimd partition_broadcast + DVE band copy, all-SBUF band-aligned muls.
  - Output: heads [128(8hx16), N] bf16 x flat W_out [128, 128] accumulated
    over the two buffers per 128-col n-tile.
"""
import numpy as np

NUM_STATION = 20
S = NUM_STATION + 1          # 21
H = 8
D = 128
K = 16
E = 128
N = 1024
B = 32
NCORES = 8
BPC = B // NCORES            # 4 batches per core
NORM = 0.25                  # 1/sqrt(16)

_CACHE = {}


def _build():
    import concourse.bass as bass
    import concourse.tile as tile
    from concourse import bacc, mybir

    F32 = mybir.dt.float32
    BF16 = mybir.dt.bfloat16
    EXP = mybir.ActivationFunctionType.Exp

    nc = bacc.Bacc("TRN2", target_bir_lowering=False, debug=False,
                   num_devices=NCORES)

    qT_d = nc.dram_tensor("qT", [BPC, D, N], F32, kind="ExternalInput").ap()
    hT_d = nc.dram_tensor("hT", [BPC, D, N], F32, kind="ExternalInput").ap()
    wnames = ["W_query_custom", "W_query_custom_1", "W_key_custom",
              "W_val_custom", "W_query_charge_1", "W_key_charge",
              "W_val_charge"]
    w_d = {n: nc.dram_tensor(n, [H, D, K], F32, kind="ExternalInput").ap()
           for n in wnames}
    wout_d = nc.dram_tensor("W_out", [H, K, E], F32, kind="ExternalInput").ap()
    out_d = nc.dram_tensor("out", [BPC, N, E], F32, kind="ExternalOutput").ap()

    with tile.TileContext(nc) as tc:
        with tc.tile_pool(name="const", bufs=1) as const, \
             tc.tile_pool(name="sb", bufs=1) as sb, \
             tc.tile_pool(name="esp", bufs=1) as esp, \
             tc.tile_pool(name="normp", bufs=1) as normp, \
             tc.tile_pool(name="ps", bufs=1, space="PSUM") as ps:

            # psum rotation over 3 two-bank slots
            scn = [0]
            def sc_tile(shape, nm):
                t = ps.tile(shape, F32, name=nm, tag=f"sc{scn[0] % 4}")
                scn[0] += 1
                return t

            # ================= weights (once per core) =================
            def flat_w(src, nm):
                tiles = []
                for X in range(2):
                    stg = const.tile([128, 128], F32, name=f"stg_{nm}{X}",
                                     tag="wstg", bufs=2)
                    nc.vector.memset(stg[:], 0.0)
                    for g in range(4):
                        nc.sync.dma_start(stg[:, 32 * g:32 * g + K],
                                          w_d[src][4 * X + g])
                    t = const.tile([128, 128], BF16, name=f"{nm}{X}")
                    nc.vector.tensor_copy(t[:], stg[:])
                    tiles.append(t)
                return tiles

            WK = flat_w("W_key_custom", "wk")
            WKC = flat_w("W_key_charge", "wkc")
            WQ1 = flat_w("W_query_custom_1", "wq1")
            WQC1 = flat_w("W_query_charge_1", "wqc1")
            WQ2 = flat_w("W_query_custom", "wq2")

            WO = []
            for X in range(2):
                stg = const.tile([128, 128], F32, name=f"wostg{X}", tag="wstg", bufs=2)
                nc.vector.memset(stg[:], 0.0)
                for g in range(4):
                    nc.scalar.dma_start(stg[32 * g:32 * g + K, :], wout_d[4 * X + g])
                t = const.tile([128, 128], BF16, name=f"wo{X}")
                nc.vector.tensor_copy(t[:], stg[:])
                WO.append(t)

            def val_w(wname, nm):
                stg = const.tile([128, 136], F32, name=f"stg_{nm}", tag="wstgv", bufs=2)
                nc.vector.memset(stg[:], 0.0)
                for h in range(H):
                    nc.scalar.dma_start(stg[:, 17 * h:17 * h + K], w_d[wname][h])
                t = const.tile([128, 136], BF16, name=nm)
                nc.vector.tensor_copy(t[:], stg[:])
                return t

            WV = val_w("W_val_custom", "wv")
            WVC = val_w("W_val_charge", "wvc")

            state = {}

            # ---------------- Phase A: loads, values, projections (128x128)
            def phase_A(b):
                st = {}
                qTf = sb.tile([128, N], F32, name=f"qTf{b}", tag="qTf")
                nc.sync.dma_start(qTf[:], qT_d[b])
                hTf = sb.tile([128, N], F32, name=f"hTf{b}", tag="qTf")
                nc.sync.dma_start(hTf[:], hT_d[b])
                qTb = sb.tile([128, N], BF16, name=f"qTb{b}", tag="qTb")
                nc.vector.tensor_copy(qTb[:], qTf[:])
                hTb = sb.tile([128, N], BF16, name=f"hTb{b}", tag="hTb")
                nc.vector.tensor_copy(hTb[:], hTf[:])

                # values Vaug[j]: [128, 160] bf16 (136 data + 24 zero pad),
                # ones at col 17h+16
                va = []
                for j in range(8):
                    pv = sc_tile([128, 136], f"pv{b}_{j}")
                    nc.tensor.matmul(pv[:], hTb[:, 128 * j:128 * j + 128], WV[:],
                                     start=True, stop=True)
                    v = sb.tile([128, 160], BF16, name=f"va{b}_{j}", tag=f"va{j}", bufs=2)
                    nc.vector.tensor_copy(v[:, 0:136], pv[:])
                    nc.vector.memset(v[:, 136:160], 0.0)
                    v3 = v[:, 0:136].rearrange("p (h s) -> p h s", h=H)
                    nc.vector.memset(v3[:, :, K:K + 1], 1.0)
                    va.append(v)
                pvs = sc_tile([128, 136], f"pvs{b}")
                nc.tensor.matmul(pvs[0:S, :], hTb[:, 0:S], WVC[:],
                                 start=True, stop=True)
                vst4 = sb.tile([128, 160], BF16, name=f"vst4{b}", tag="vst4", bufs=2)
                nc.vector.tensor_copy(vst4[0:S, 0:136], pvs[0:S, :])
                nc.vector.memset(vst4[0:S, 136:160], 0.0)
                vst3 = vst4[0:S, 0:136].rearrange("p (h s) -> p h s", h=H)
                nc.vector.memset(vst3[:, :, K:K + 1], 1.0)
                for g in range(1, 4):
                    nc.vector.tensor_copy(vst4[32 * g:32 * g + S, :], vst4[0:S, :])

                kt, q1, q2 = [], [], []
                for X in range(2):
                    pk = sc_tile([128, N], f"pk{b}_{X}")
                    nc.tensor.matmul(pk[:, 0:S], WKC[X][:], hTb[:, 0:S],
                                     start=True, stop=True)
                    nc.tensor.matmul(pk[:, S:512], WK[X][:], hTb[:, S:512],
                                     start=True, stop=True)
                    nc.tensor.matmul(pk[:, 512:N], WK[X][:], hTb[:, 512:N],
                                     start=True, stop=True)
                    k_ = sb.tile([128, N], BF16, name=f"kt{b}_{X}", tag=f"kt{X}")
                    nc.vector.tensor_copy(k_[:], pk[:])
                    kt.append(k_)
                    p1 = sc_tile([128, N], f"p1{b}_{X}")
                    nc.tensor.matmul(p1[:, 0:S], WQC1[X][:], qTb[:, 0:S],
                                     start=True, stop=True)
                    nc.tensor.matmul(p1[:, S:512], WQ1[X][:], qTb[:, S:512],
                                     start=True, stop=True)
                    nc.tensor.matmul(p1[:, 512:N], WQ1[X][:], qTb[:, 512:N],
                                     start=True, stop=True)
                    q1_ = sb.tile([128, N], BF16, name=f"q1{b}_{X}", tag=f"q1{X}")
                    nc.vector.tensor_copy(q1_[:], p1[:])
                    q1.append(q1_)
                    p2 = sc_tile([128, N], f"p2{b}_{X}")
                    nc.tensor.matmul(p2[:, 0:512], WQ2[X][:], qTb[:, 0:512],
                                     start=True, stop=True)
                    nc.tensor.matmul(p2[:, 512:N], WQ2[X][:], qTb[:, 512:N],
                                     start=True, stop=True)
                    q2_ = sb.tile([128, N], BF16, name=f"q2{b}_{X}", tag=f"q2{X}")
                    nc.vector.tensor_copy(q2_[:], p2[:])
                    q2.append(q2_)
                st["kt"], st["q1"], st["q2"] = kt, q1, q2
                st["va"], st["vst4"] = va, vst4
                return st

            # ------- Phase B tile: tt-scores for (X, j) (32x128 row tiling)
            def phase_B_tile(b, st, X, j):
                es = st.setdefault("es", {})
                kt, q1 = st["kt"], st["q1"]
                for g in range(4):
                    h = 4 * X + g
                    stp = sc_tile([128, N], f"st{b}_{h}_{j}")
                    lhs = kt[X][32 * g:32 * g + K, 128 * j:128 * j + 128]
                    nc.tensor.matmul(stp[:, 0:512], lhs,
                                     q1[X][32 * g:32 * g + K, 0:512],
                                     start=True, stop=True,
                                     tile_position=(32 * g, 0))
                    nc.tensor.matmul(stp[:, 512:N], lhs,
                                     q1[X][32 * g:32 * g + K, 512:N],
                                     start=True, stop=True,
                                     tile_position=(32 * g, 0))
                    e_ = esp.tile([128, N], BF16, name=f"es{b}_{h}_{j}",
                                  tag=f"es{h}_{j}")
                    nc.scalar.activation(e_[:], stp[:], EXP, scale=NORM)
                    if j == 0:
                        nc.vector.memset(e_[0:S, :], 0.0)
                    es[(h, j)] = e_

            # ------- Phase TS scores(X): ts-scores (32x32)
            def phase_TS_scores(b, st, X):
                kt, q2 = st["kt"], st["q2"]
                ps2 = sc_tile([128, N], f"ps2{b}_{X}")
                for g in range(4):
                    lhs2 = kt[X][32 * g:32 * g + K, 0:S]
                    for half in range(2):
                        nc.tensor.matmul(
                            ps2[32 * g:32 * g + S, 512 * half:512 * half + 512],
                            lhs2,
                            q2[X][32 * g:32 * g + K, 512 * half:512 * half + 512],
                            start=True, stop=True,
                            tile_position=(32 * g, 32 * g))
                e2 = sb.tile([128, N], BF16, name=f"es2{b}_{X}", tag="es2")
                nc.scalar.activation(e2[:], ps2[:], EXP, scale=NORM)
                st.setdefault("es2", {})[X] = e2

            # ------- Phase TS pts(X): station AV + s-side norm prep (32x32)
            def phase_TS_pts(b, st, X):
                vst4 = st["vst4"]
                e2 = st["es2"][X]
                rp = sb.tile([128, N], F32, name=f"rawpts{b}_{X}", tag=f"rpts{X}")
                for half in range(2):
                    pts = sc_tile([128, 512], f"pts{b}_{X}_{half}")
                    for g in range(4):
                        h = 4 * X + g
                        nc.tensor.matmul(
                            pts[32 * g:32 * g + 32, :],
                            vst4[32 * g:32 * g + S, 17 * h:17 * h + 32],
                            e2[32 * g:32 * g + S, 512 * half:512 * half + 512],
                            start=True, stop=True,
                            tile_position=(32 * g, 32 * g))
                    nc.vector.tensor_copy(rp[:, 512 * half:512 * half + 512],
                                          pts[:])
                st.setdefault("rawpts", {})[X] = rp
                rbs = sb.tile([128, N], F32, name=f"rbs{b}_{X}", tag=f"rbs{X}")
                nc.vector.memset(rbs[:], 1.0)
                for g in range(4):
                    srow = normp.tile([1, N], F32, name=f"srs{b}_{X}_{g}",
                                      tag="srow", bufs=2)
                    nc.sync.dma_start(srow[:], rp[32 * g + K:32 * g + 17, :])
                    rb16 = normp.tile([16, N], F32, name=f"rb16s{b}_{X}_{g}",
                                      tag="rb16")
                    nc.gpsimd.partition_broadcast(rb16[:], srow[:])
                    nc.vector.tensor_copy(rbs[32 * g:32 * g + K, :], rb16[:])
                nc.vector.reciprocal_approx_fast(rbs[:], rbs[:])
                st.setdefault("rbs", {})[X] = rbs

            # ------- Phase C (AV, 128x32 col tiling): j-pair psums through
            # the sc rotation, accumulated into the SBUF raw tile by DVE
            def phase_C(b, st, X):
                va, es = st["va"], st["es"]
                ra = sb.tile([128, N], F32, name=f"rawav{b}_{X}", tag=f"rav{X}")
                for half in range(2):
                    hs = slice(512 * half, 512 * half + 512)
                    for jp in range(4):
                        pav = sc_tile([128, 512], f"pav{b}_{X}_{half}_{jp}")
                        for jj in range(2):
                            j = 2 * jp + jj
                            for g in range(4):
                                h = 4 * X + g
                                nc.tensor.matmul(
                                    pav[32 * g:32 * g + 32, :],
                                    va[j][:, 17 * h:17 * h + 32],
                                    es[(h, j)][:, hs],
                                    start=(jj == 0), stop=(jj == 1),
                                    tile_position=(0, 32 * g),
                                    skip_group_check=True)
                        if jp == 0:
                            nc.vector.tensor_copy(ra[:, hs], pav[:])
                        else:
                            nc.vector.tensor_add(ra[:, hs], ra[:, hs], pav[:])
                st.setdefault("rawav", {})[X] = ra

            # ------- Phase NT(X): t-side norm + heads assembly (no PE)
            def phase_NT(b, st, X):
                ra, rp = st["rawav"][X], st["rawpts"][X]
                rbs = st["rbs"][X]
                rbt = sb.tile([128, N], F32, name=f"rbt{b}_{X}", tag="rbt")
                if b == 0 and X == 0:
                    nc.vector.memset(rbt[:], 1.0)
                for g in range(4):
                    srow = normp.tile([1, N], F32, name=f"srt{b}_{X}_{g}",
                                      tag="srow", bufs=2)
                    nc.sync.dma_start(srow[:], ra[32 * g + K:32 * g + 17, :])
                    rb16 = normp.tile([16, N], F32, name=f"rb16t{b}_{X}_{g}",
                                      tag="rb16")
                    nc.gpsimd.partition_broadcast(rb16[:], srow[:])
                    nc.vector.tensor_copy(rbt[32 * g:32 * g + K, :], rb16[:])
                nc.vector.reciprocal_approx_fast(rbt[:], rbt[:])
                hx = sb.tile([128, N], BF16, name=f"heads{b}_{X}",
                             tag=f"heads{X}")
                nc.vector.tensor_mul(hx[:], ra[:], rbt[:])
                nc.vector.tensor_mul(rbs[:, S:N], rp[:, S:N], rbs[:, S:N])
                nc.vector.tensor_add(hx[:, S:N], hx[:, S:N], rbs[:, S:N])
                st.setdefault("heads", {})[X] = hx

            # ---------------- Phase D: output projection (128x128)
            def phase_D(b, st):
                heads = st["heads"]
                for nt in range(8):
                    po = sc_tile([128, 128], f"po{b}_{nt}")
                    nc.tensor.matmul(po[:], heads[0][:, 128 * nt:128 * nt + 128],
                                     WO[0][:], start=True, stop=False,
                                     skip_group_check=True)
                    nc.tensor.matmul(po[:], heads[1][:, 128 * nt:128 * nt + 128],
                                     WO[1][:], start=False, stop=True,
                                     skip_group_check=True)
                    ot = normp.tile([128, 128], F32, name=f"ot{b}_{nt}", tag="ot")
                    nc.vector.tensor_copy(ot[:], po[:])
                    nc.sync.dma_start(out_d[b, 128 * nt:128 * nt + 128, :], ot[:])

            # ---------------- software-pipelined emission
            states = {}
            states[0] = phase_A(0)
            phase_TS_scores(0, states[0], 0)
            phase_TS_scores(0, states[0], 1)
            for b in range(BPC):
                st = states[b]
                phase_TS_pts(b, st, 0)
                phase_TS_pts(b, st, 1)
                for j in range(8):
                    phase_B_tile(b, st, 0, j)
                for j in range(8):
                    phase_B_tile(b, st, 1, j)
                if b > 0:
                    phase_D(b - 1, states[b - 1])
                    del states[b - 1]
                if b + 1 < BPC:
                    states[b + 1] = phase_A(b + 1)
                    phase_TS_scores(b + 1, states[b + 1], 0)
                    phase_TS_scores(b + 1, states[b + 1], 1)
                phase_C(b, st, 0)
                phase_NT(b, st, 0)
                phase_C(b, st, 1)
                phase_NT(b, st, 1)
            phase_D(BPC - 1, states[BPC - 1])

    nc.compile()
    return nc


def _get_nc():
    if "nc" not in _CACHE:
        _CACHE["nc"] = _build()
    return _CACHE["nc"]


def _kernel_jax(q, h, Ws):
    """Batch-sharded (data-parallel) attention on the 8 NeuronCores via pmap."""
    import jax, jax.numpy as jnp
    S_ = S
    NORMc = np.float32(NORM)

    def one_shard(q, h, W_query_custom, W_query_custom_1, W_key_custom,
                  W_val_custom, W_query_charge_1, W_key_charge, W_val_charge,
                  W_out):
        h_st, h_tk = h[:, :S_], h[:, S_:]
        q_st, q_tk = q[:, :S_], q[:, S_:]
        proj = lambda x, W: jnp.einsum('bnd,hdk->hbnk', x, W)
        K_c = proj(h_tk, W_key_custom)
        V_c = proj(h_tk, W_val_custom)
        K_s = proj(h_st, W_key_charge)
        V_s = proj(h_st, W_val_charge)
        Q_tt = proj(q_tk, W_query_custom_1)
        A_tt = jax.nn.softmax(NORMc * jnp.einsum('hbqk,hbtk->hbqt', Q_tt, K_c), axis=-1)
        heads_t = jnp.einsum('hbqt,hbtk->hbqk', A_tt, V_c)
        Q_ts = proj(q_tk, W_query_custom)
        A_ts = jax.nn.softmax(NORMc * jnp.einsum('hbqk,hbsk->hbqs', Q_ts, K_s), axis=-1)
        heads_t = heads_t + jnp.einsum('hbqs,hbsk->hbqk', A_ts, V_s)
        Q_st = proj(q_st, W_query_charge_1)
        A_st = jax.nn.softmax(NORMc * jnp.einsum('hbqk,hbtk->hbqt', Q_st, K_c), axis=-1)
        heads_s = jnp.einsum('hbqt,hbtk->hbqk', A_st, V_c)
        heads = jnp.concatenate([heads_s, heads_t], axis=2)
        return jnp.einsum('hbnk,hke->bne', heads, W_out)

    if "pmap_fn" not in _CACHE:
        _CACHE["pmap_fn"] = jax.pmap(one_shard, axis_name="i")
    f = _CACHE["pmap_fn"]
    qs = q.reshape(NCORES, BPC, N, D)
    hs = h.reshape(NCORES, BPC, N, D)
    wkey = tuple(w.tobytes()[:64] for w in Ws)
    if _CACHE.get("wkey") != wkey:
        _CACHE["wrep"] = [jax.device_put_replicated(jnp.asarray(w), jax.devices()[:NCORES])
                          for w in Ws]
        _CACHE["wkey"] = wkey
    out = f(qs, hs, *_CACHE["wrep"])
    return np.asarray(out).reshape(B, N, E)


USE_BASS = True


def kernel(q, h, W_query_custom, W_query_custom_1, W_key_custom, W_val_custom,
           W_query_charge_1, W_key_charge, W_val_charge, W_out, _trace=False):
    if not USE_BASS:
        Ws = [np.asarray(w, np.float32) for w in
              (W_query_custom, W_query_custom_1, W_key_custom, W_val_custom,
               W_query_charge_1, W_key_charge, W_val_charge, W_out)]
        return _kernel_jax(np.asarray(q, np.float32), np.asarray(h, np.float32), Ws)
    return _kernel_bass(q, h, W_query_custom, W_query_custom_1, W_key_custom,
                        W_val_custom, W_query_charge_1, W_key_charge,
                        W_val_charge, W_out, _trace)


def _kernel_bass(q, h, W_query_custom, W_query_custom_1, W_key_custom, W_val_custom,
                 W_query_charge_1, W_key_charge, W_val_charge, W_out, _trace=False):
    from concourse.bass_utils import run_bass_kernel_spmd

    nc = _get_nc()
    qT = np.ascontiguousarray(np.asarray(q, dtype=np.float32).transpose(0, 2, 1))
    hT = np.ascontiguousarray(np.asarray(h, dtype=np.float32).transpose(0, 2, 1))
    ws = {
        "W_query_custom": W_query_custom, "W_query_custom_1": W_query_custom_1,
        "W_key_custom": W_key_custom, "W_val_custom": W_val_custom,
        "W_query_charge_1": W_query_charge_1, "W_key_charge": W_key_charge,
        "W_val_charge": W_val_charge, "W_out": W_out,
    }
    ws = {k: np.ascontiguousarray(np.asarray(v, dtype=np.float32))
          for k, v in ws.items()}
    in_maps = []
    for c in range(NCORES):
        m = {"qT": qT[c * BPC:(c + 1) * BPC], "hT": hT[c * BPC:(c + 1) * BPC]}
        m.update(ws)
        in_maps.append(m)
    res = run_bass_kernel_spmd(nc, in_maps, core_ids=list(range(NCORES)),
                               trace=_trace)
    out = np.concatenate([res.results[c]["out"] for c in range(NCORES)], axis=0)
    if _trace:
        _CACHE["last_results"] = res
    return out


# revision 27
# speedup vs baseline: 1.4142x; 1.0032x over previous
"""Trainium2 Bass kernel for nn_HMHA (heterogeneous multi-head attention).

Reference semantics (B=32, N=1024, D=128, H=8, K=16, S=21 stations, T=1003 tasks):
  - 7 per-head projections of q/h slices, three attention blocks
    (task->task, task->station, station->task), all softmaxed over keys,
    combined and projected by W_out.

Sharding: data-parallel over batch across 8 cores (4 batches/core).

v2 layout (per core, per batch), all PE inputs bf16:
  - Heads split into two buffers: A = heads 0-3, B = heads 4-7, head g of a
    buffer at 32-aligned partition band 32g (PE tile_position row/col bands).
  - Flat projections: kt/q1/q2 [128(4hx32band), N] via single [128,128] flat
    weight matmuls (cols 0:21 use the charge/station weights, 21: the task
    weights, matching position-dependent projection in the reference).
  - tt-scores: row-tiled (32x128 mode) matmuls, 2 heads concurrent,
    psum [128keys, N] -> exp (scalar ACT, scale 0.25) -> es bf16 SBUF.
  - ts-scores + station AV: 32x32 diagonal tiles, 4 heads packed per psum.
  - AV: col-tiled (128x32 mode), 4 heads packed into [128, 512] psum halves,
    Vaug carries a ones-slot per head -> row 32g+16 = softmax denominator.
  - Normalization: denominators DMAd to [1,N] tiles, reciprocal on DVE,
    gpsimd partition_broadcast + DVE band copy, all-SBUF band-aligned muls.
  - Output: heads [128(8hx16), N] bf16 x flat W_out [128, 128] accumulated
    over the two buffers per 128-col n-tile.
"""
import numpy as np

NUM_STATION = 20
S = NUM_STATION + 1          # 21
H = 8
D = 128
K = 16
E = 128
N = 1024
B = 32
NCORES = 8
BPC = B // NCORES            # 4 batches per core
NORM = 0.25                  # 1/sqrt(16)

_CACHE = {}


def _build():
    import concourse.bass as bass
    import concourse.tile as tile
    from concourse import bacc, mybir

    F32 = mybir.dt.float32
    F32R = mybir.dt.float32r
    BF16 = mybir.dt.bfloat16
    EXP = mybir.ActivationFunctionType.Exp

    nc = bacc.Bacc("TRN2", target_bir_lowering=False, debug=False,
                   num_devices=NCORES)

    qT_d = nc.dram_tensor("qT", [BPC, D, N], F32, kind="ExternalInput").ap()
    hT_d = nc.dram_tensor("hT", [BPC, D, N], F32, kind="ExternalInput").ap()
    wnames = ["W_query_custom", "W_query_custom_1", "W_key_custom",
              "W_val_custom", "W_query_charge_1", "W_key_charge",
              "W_val_charge"]
    w_d = {n: nc.dram_tensor(n, [H, D, K], F32, kind="ExternalInput").ap()
           for n in wnames}
    wout_d = nc.dram_tensor("W_out", [H, K, E], F32, kind="ExternalInput").ap()
    out_d = nc.dram_tensor("out", [BPC, N, E], F32, kind="ExternalOutput").ap()

    with tile.TileContext(nc) as tc:
        with tc.tile_pool(name="const", bufs=1) as const, \
             tc.tile_pool(name="sb", bufs=1) as sb, \
             tc.tile_pool(name="esp", bufs=1) as esp, \
             tc.tile_pool(name="normp", bufs=1) as normp, \
             tc.tile_pool(name="ps", bufs=1, space="PSUM") as ps:

            # psum rotation over 3 two-bank slots
            scn = [0]
            def sc_tile(shape, nm):
                t = ps.tile(shape, F32, name=nm, tag=f"sc{scn[0] % 4}")
                scn[0] += 1
                return t

            # ================= weights (once per core) =================
            def flat_w(src, nm):
                tiles = []
                for X in range(2):
                    stg = const.tile([128, 128], F32, name=f"stg_{nm}{X}",
                                     tag="wstg", bufs=2)
                    nc.vector.memset(stg[:], 0.0)
                    for g in range(4):
                        nc.sync.dma_start(stg[:, 32 * g:32 * g + K],
                                          w_d[src][4 * X + g])
                    t = const.tile([128, 128], BF16, name=f"{nm}{X}")
                    nc.vector.tensor_copy(t[:], stg[:])
                    tiles.append(t)
                return tiles

            WK = flat_w("W_key_custom", "wk")
            WKC = flat_w("W_key_charge", "wkc")
            WQ1 = flat_w("W_query_custom_1", "wq1")
            WQC1 = flat_w("W_query_charge_1", "wqc1")
            WQ2 = flat_w("W_query_custom", "wq2")

            WO = []
            for X in range(2):
                stg = const.tile([128, 128], F32, name=f"wostg{X}", tag="wstg", bufs=2)
                nc.vector.memset(stg[:], 0.0)
                for g in range(4):
                    nc.scalar.dma_start(stg[32 * g:32 * g + K, :], wout_d[4 * X + g])
                t = const.tile([128, 128], BF16, name=f"wo{X}")
                nc.vector.tensor_copy(t[:], stg[:])
                WO.append(t)

            def val_w(wname, nm):
                stg = const.tile([128, 256], F32, name=f"stg_{nm}", tag="wstgv", bufs=2)
                nc.vector.memset(stg[:], 0.0)
                for h in range(H):
                    nc.scalar.dma_start(stg[:, 32 * h:32 * h + K], w_d[wname][h])
                t = const.tile([128, 256], BF16, name=nm)
                nc.vector.tensor_copy(t[:], stg[:])
                return t

            WV = val_w("W_val_custom", "wv")
            WVC = val_w("W_val_charge", "wvc")


            state = {}

            # ---------------- Phase A: loads, values, projections (128x128)
            def phase_A(b):
                st = {}
                qTf = sb.tile([128, N], F32, name=f"qTf{b}", tag="qTf")
                nc.sync.dma_start(qTf[:], qT_d[b])
                hTf = sb.tile([128, N], F32, name=f"hTf{b}", tag="qTf")
                nc.sync.dma_start(hTf[:], hT_d[b])
                qTb = sb.tile([128, N], BF16, name=f"qTb{b}", tag="qTb")
                nc.vector.tensor_copy(qTb[:], qTf[:])
                hTb = sb.tile([128, N], BF16, name=f"hTb{b}", tag="hTb")
                nc.vector.tensor_copy(hTb[:], hTf[:])

                # values Vaug[j]: [128, 160] bf16 (136 data + 24 zero pad),
                # ones at col 17h+16
                va = []
                for j in range(8):
                    pv = sc_tile([128, 256], f"pv{b}_{j}")
                    nc.tensor.matmul(pv[:], hTb[:, 128 * j:128 * j + 128], WV[:],
                                     start=True, stop=True)
                    v = sb.tile([128, 256], BF16, name=f"va{b}_{j}", tag=f"va{j}", bufs=2)
                    nc.vector.tensor_copy(v[:], pv[:])
                    v3 = v[:].rearrange("p (h s) -> p h s", h=H)
                    nc.vector.memset(v3[:, :, K:32], 1.0)
                    va.append(v)
                pvs = sc_tile([128, 256], f"pvs{b}")
                nc.tensor.matmul(pvs[0:S, :], hTb[:, 0:S], WVC[:],
                                 start=True, stop=True)
                vst4 = sb.tile([128, 256], BF16, name=f"vst4{b}", tag="vst4", bufs=2)
                nc.vector.tensor_copy(vst4[0:S, :], pvs[0:S, :])
                vst3 = vst4[0:S, :].rearrange("p (h s) -> p h s", h=H)
                nc.vector.memset(vst3[:, :, K:32], 1.0)
                for g in range(1, 4):
                    nc.vector.tensor_copy(vst4[32 * g:32 * g + S, :], vst4[0:S, :])

                kt, q1, q2 = [], [], []
                for X in range(2):
                    pk = sc_tile([128, N], f"pk{b}_{X}")
                    nc.tensor.matmul(pk[:, 0:S], WKC[X][:], hTb[:, 0:S],
                                     start=True, stop=True)
                    nc.tensor.matmul(pk[:, S:512], WK[X][:], hTb[:, S:512],
                                     start=True, stop=True)
                    nc.tensor.matmul(pk[:, 512:N], WK[X][:], hTb[:, 512:N],
                                     start=True, stop=True)
                    k_ = sb.tile([128, N], BF16, name=f"kt{b}_{X}", tag=f"kt{X}")
                    nc.vector.tensor_copy(k_[:], pk[:])
                    kt.append(k_)
                    p1 = sc_tile([128, N], f"p1{b}_{X}")
                    nc.tensor.matmul(p1[:, 0:S], WQC1[X][:], qTb[:, 0:S],
                                     start=True, stop=True)
                    nc.tensor.matmul(p1[:, S:512], WQ1[X][:], qTb[:, S:512],
                                     start=True, stop=True)
                    nc.tensor.matmul(p1[:, 512:N], WQ1[X][:], qTb[:, 512:N],
                                     start=True, stop=True)
                    q1_ = sb.tile([128, N], BF16, name=f"q1{b}_{X}", tag=f"q1{X}")
                    nc.vector.tensor_copy(q1_[:], p1[:])
                    q1.append(q1_)
                    p2 = sc_tile([128, N], f"p2{b}_{X}")
                    nc.tensor.matmul(p2[:, 0:512], WQ2[X][:], qTb[:, 0:512],
                                     start=True, stop=True)
                    nc.tensor.matmul(p2[:, 512:N], WQ2[X][:], qTb[:, 512:N],
                                     start=True, stop=True)
                    q2_ = sb.tile([128, N], BF16, name=f"q2{b}_{X}", tag=f"q2{X}")
                    nc.vector.tensor_copy(q2_[:], p2[:])
                    q2.append(q2_)
                st["kt"], st["q1"], st["q2"] = kt, q1, q2
                st["va"], st["vst4"] = va, vst4
                return st

            # ------- Phase B tile: tt-scores for (X, j) (32x128 row tiling)
            def phase_B_tile(b, st, X, j):
                es = st.setdefault("es", {})
                kt, q1 = st["kt"], st["q1"]
                for g in range(4):
                    h = 4 * X + g
                    stp = sc_tile([128, N], f"st{b}_{h}_{j}")
                    lhs = kt[X][32 * g:32 * g + K, 128 * j:128 * j + 128]
                    nc.tensor.matmul(stp[:, 0:512], lhs,
                                     q1[X][32 * g:32 * g + K, 0:512],
                                     start=True, stop=True,
                                     tile_position=(32 * g, 0))
                    nc.tensor.matmul(stp[:, 512:N], lhs,
                                     q1[X][32 * g:32 * g + K, 512:N],
                                     start=True, stop=True,
                                     tile_position=(32 * g, 0))
                    e_ = esp.tile([128, N], BF16, name=f"es{b}_{h}_{j}",
                                  tag=f"es{h}_{j}")
                    nc.scalar.activation(e_[:], stp[:], EXP, scale=NORM)
                    if j == 0:
                        nc.vector.memset(e_[0:S, :], 0.0)
                    es[(h, j)] = e_

            # ------- Phase TS scores(X): ts-scores (32x32)
            def phase_TS_scores(b, st, X):
                kt, q2 = st["kt"], st["q2"]
                ps2 = sc_tile([128, N], f"ps2{b}_{X}")
                for g in range(4):
                    lhs2 = kt[X][32 * g:32 * g + K, 0:S]
                    for half in range(2):
                        nc.tensor.matmul(
                            ps2[32 * g:32 * g + S, 512 * half:512 * half + 512],
                            lhs2,
                            q2[X][32 * g:32 * g + K, 512 * half:512 * half + 512],
                            start=True, stop=True,
                            tile_position=(32 * g, 32 * g))
                e2 = sb.tile([128, N], BF16, name=f"es2{b}_{X}", tag="es2", bufs=2)
                nc.scalar.activation(e2[:], ps2[:], EXP, scale=NORM)
                st.setdefault("es2", {})[X] = e2

            # ------- Phase TS pts(X): station AV + s-side norm prep (32x32)
            def phase_TS_pts(b, st, X):
                vst4 = st["vst4"]
                e2 = st["es2"][X]
                rp = sb.tile([128, N], F32, name=f"rawpts{b}_{X}", tag=f"rpts{X}")
                for half in range(2):
                    pts = sc_tile([128, 512], f"pts{b}_{X}_{half}")
                    for g in range(4):
                        h = 4 * X + g
                        nc.tensor.matmul(
                            pts[32 * g:32 * g + 32, :],
                            vst4[32 * g:32 * g + S, 32 * h:32 * h + 32],
                            e2[32 * g:32 * g + S, 512 * half:512 * half + 512],
                            start=True, stop=True,
                            tile_position=(32 * g, 32 * g))
                    nc.vector.tensor_copy(rp[:, 512 * half:512 * half + 512],
                                          pts[:])
                st.setdefault("rawpts", {})[X] = rp
                rbs = sb.tile([128, N], F32, name=f"rbs{b}_{X}", tag=f"rbs{X}")
                if b == 0:
                    nc.vector.memset(rbs[:], 1.0)
                for g in range(4):
                    nc.sync.dma_start(rbs[32 * g:32 * g + K, :],
                                      rp[32 * g + K:32 * g + 32, :])
                nc.vector.reciprocal_approx_fast(rbs[:], rbs[:])
                st.setdefault("rbs", {})[X] = rbs

            # ------- Phase C (AV, 128x32 col tiling): j-pair psums through
            # the sc rotation, accumulated into the SBUF raw tile by DVE
            def phase_C(b, st, X):
                va, es = st["va"], st["es"]
                ra = sb.tile([128, N], F32, name=f"rawav{b}_{X}", tag=f"rav{X}")
                for half in range(2):
                    hs = slice(512 * half, 512 * half + 512)
                    for jp in range(4):
                        pav = sc_tile([128, 512], f"pav{b}_{X}_{half}_{jp}")
                        for jj in range(2):
                            j = 2 * jp + jj
                            for g in range(4):
                                h = 4 * X + g
                                nc.tensor.matmul(
                                    pav[32 * g:32 * g + 32, :],
                                    va[j][:, 32 * h:32 * h + 32],
                                    es[(h, j)][:, hs],
                                    start=(jj == 0), stop=(jj == 1),
                                    tile_position=(0, 32 * g),
                                    skip_group_check=True)
                        if jp == 0:
                            nc.vector.tensor_copy(ra[:, hs], pav[:])
                        else:
                            nc.vector.tensor_add(ra[:, hs], ra[:, hs], pav[:])
                st.setdefault("rawav", {})[X] = ra

            # ------- Phase NT(X): t-side norm + heads assembly
            def phase_NT(b, st, X):
                ra, rp = st["rawav"][X], st["rawpts"][X]
                rbs = st["rbs"][X]
                rbt = sb.tile([128, N], F32, name=f"rbt{b}_{X}", tag="rbt")
                if b == 0 and X == 0:
                    nc.vector.memset(rbt[:], 1.0)
                for g in range(4):
                    nc.sync.dma_start(rbt[32 * g:32 * g + K, :],
                                      ra[32 * g + K:32 * g + 32, :])
                nc.vector.reciprocal_approx_fast(rbt[:], rbt[:])
                hx = sb.tile([128, N], BF16, name=f"heads{b}_{X}",
                             tag=f"heads{X}")
                nc.vector.tensor_mul(hx[:], ra[:], rbt[:])
                nc.vector.tensor_mul(rbs[:, S:N], rp[:, S:N], rbs[:, S:N])
                nc.vector.tensor_add(hx[:, S:N], hx[:, S:N], rbs[:, S:N])
                st.setdefault("heads", {})[X] = hx

            # ---------------- Phase D: output projection (128x128)
            def phase_D(b, st):
                heads = st["heads"]
                for nt in range(8):
                    po = sc_tile([128, 128], f"po{b}_{nt}")
                    nc.tensor.matmul(po[:], heads[0][:, 128 * nt:128 * nt + 128],
                                     WO[0][:], start=True, stop=False,
                                     skip_group_check=True)
                    nc.tensor.matmul(po[:], heads[1][:, 128 * nt:128 * nt + 128],
                                     WO[1][:], start=False, stop=True,
                                     skip_group_check=True)
                    ot = normp.tile([128, 128], F32, name=f"ot{b}_{nt}", tag="ot")
                    nc.vector.tensor_copy(ot[:], po[:])
                    nc.sync.dma_start(out_d[b, 128 * nt:128 * nt + 128, :], ot[:])

            # ---------------- software-pipelined emission
            states = {}
            states[0] = phase_A(0)
            phase_TS_scores(0, states[0], 0)
            phase_TS_scores(0, states[0], 1)
            for b in range(BPC):
                st = states[b]
                phase_TS_pts(b, st, 0)
                phase_TS_pts(b, st, 1)
                for j in range(8):
                    phase_B_tile(b, st, 0, j)
                for j in range(8):
                    phase_B_tile(b, st, 1, j)
                if b > 0:
                    phase_D(b - 1, states[b - 1])
                    del states[b - 1]
                if b + 1 < BPC:
                    states[b + 1] = phase_A(b + 1)
                    phase_TS_scores(b + 1, states[b + 1], 0)
                    phase_TS_scores(b + 1, states[b + 1], 1)
                phase_C(b, st, 0)
                phase_NT(b, st, 0)
                phase_C(b, st, 1)
                phase_NT(b, st, 1)
            phase_D(BPC - 1, states[BPC - 1])

    nc.compile()
    return nc


def _get_nc():
    if "nc" not in _CACHE:
        _CACHE["nc"] = _build()
    return _CACHE["nc"]


def _kernel_jax(q, h, Ws):
    """Batch-sharded (data-parallel) attention on the 8 NeuronCores via pmap."""
    import jax, jax.numpy as jnp
    S_ = S
    NORMc = np.float32(NORM)

    def one_shard(q, h, W_query_custom, W_query_custom_1, W_key_custom,
                  W_val_custom, W_query_charge_1, W_key_charge, W_val_charge,
                  W_out):
        h_st, h_tk = h[:, :S_], h[:, S_:]
        q_st, q_tk = q[:, :S_], q[:, S_:]
        proj = lambda x, W: jnp.einsum('bnd,hdk->hbnk', x, W)
        K_c = proj(h_tk, W_key_custom)
        V_c = proj(h_tk, W_val_custom)
        K_s = proj(h_st, W_key_charge)
        V_s = proj(h_st, W_val_charge)
        Q_tt = proj(q_tk, W_query_custom_1)
        A_tt = jax.nn.softmax(NORMc * jnp.einsum('hbqk,hbtk->hbqt', Q_tt, K_c), axis=-1)
        heads_t = jnp.einsum('hbqt,hbtk->hbqk', A_tt, V_c)
        Q_ts = proj(q_tk, W_query_custom)
        A_ts = jax.nn.softmax(NORMc * jnp.einsum('hbqk,hbsk->hbqs', Q_ts, K_s), axis=-1)
        heads_t = heads_t + jnp.einsum('hbqs,hbsk->hbqk', A_ts, V_s)
        Q_st = proj(q_st, W_query_charge_1)
        A_st = jax.nn.softmax(NORMc * jnp.einsum('hbqk,hbtk->hbqt', Q_st, K_c), axis=-1)
        heads_s = jnp.einsum('hbqt,hbtk->hbqk', A_st, V_c)
        heads = jnp.concatenate([heads_s, heads_t], axis=2)
        return jnp.einsum('hbnk,hke->bne', heads, W_out)

    if "pmap_fn" not in _CACHE:
        _CACHE["pmap_fn"] = jax.pmap(one_shard, axis_name="i")
    f = _CACHE["pmap_fn"]
    qs = q.reshape(NCORES, BPC, N, D)
    hs = h.reshape(NCORES, BPC, N, D)
    wkey = tuple(w.tobytes()[:64] for w in Ws)
    if _CACHE.get("wkey") != wkey:
        _CACHE["wrep"] = [jax.device_put_replicated(jnp.asarray(w), jax.devices()[:NCORES])
                          for w in Ws]
        _CACHE["wkey"] = wkey
    out = f(qs, hs, *_CACHE["wrep"])
    return np.asarray(out).reshape(B, N, E)


USE_BASS = True


def kernel(q, h, W_query_custom, W_query_custom_1, W_key_custom, W_val_custom,
           W_query_charge_1, W_key_charge, W_val_charge, W_out, _trace=False):
    if not USE_BASS:
        Ws = [np.asarray(w, np.float32) for w in
              (W_query_custom, W_query_custom_1, W_key_custom, W_val_custom,
               W_query_charge_1, W_key_charge, W_val_charge, W_out)]
        return _kernel_jax(np.asarray(q, np.float32), np.asarray(h, np.float32), Ws)
    return _kernel_bass(q, h, W_query_custom, W_query_custom_1, W_key_custom,
                        W_val_custom, W_query_charge_1, W_key_charge,
                        W_val_charge, W_out, _trace)


def _kernel_bass(q, h, W_query_custom, W_query_custom_1, W_key_custom, W_val_custom,
                 W_query_charge_1, W_key_charge, W_val_charge, W_out, _trace=False):
    from concourse.bass_utils import run_bass_kernel_spmd

    nc = _get_nc()
    qT = np.ascontiguousarray(np.asarray(q, dtype=np.float32).transpose(0, 2, 1))
    hT = np.ascontiguousarray(np.asarray(h, dtype=np.float32).transpose(0, 2, 1))
    ws = {
        "W_query_custom": W_query_custom, "W_query_custom_1": W_query_custom_1,
        "W_key_custom": W_key_custom, "W_val_custom": W_val_custom,
        "W_query_charge_1": W_query_charge_1, "W_key_charge": W_key_charge,
        "W_val_charge": W_val_charge, "W_out": W_out,
    }
    ws = {k: np.ascontiguousarray(np.asarray(v, dtype=np.float32))
          for k, v in ws.items()}
    in_maps = []
    for c in range(NCORES):
        m = {"qT": qT[c * BPC:(c + 1) * BPC], "hT": hT[c * BPC:(c + 1) * BPC]}
        m.update(ws)
        in_maps.append(m)
    res = run_bass_kernel_spmd(nc, in_maps, core_ids=list(range(NCORES)),
                               trace=_trace)
    out = np.concatenate([res.results[c]["out"] for c in range(NCORES)], axis=0)
    if _trace:
        _CACHE["last_results"] = res
    return out
